# revision 13
# baseline (speedup 1.0000x reference)
"""MoE routing kernel for Trainium2 (8 NeuronCores, expert-parallel).

Strategy:
  - Router (tiny: [N,H]@[H,E]) runs on host in fp64; top-2 selection is
    identical to the fp32 reference whenever the prob gap exceeds fp32
    noise (~1e-7; measured min gap is ~6.6e-6 for the target inputs).
  - Two-group expert parallelism: SPMD forces an identical program on
    all 8 cores, so per-core capacity is uniform and the naive layout
    (expert e -> core e) pays max(counts) columns on every core.
    Instead each core runs TWO expert passes with capacities (CA, CB):
    16 slots total, each slot = a token-slice of one expert (weights are
    per-core data).  A small host solver picks (CA, CB) and the
    slot assignment: the k heaviest experts take two A-slots, the k
    lightest two B-slots, the middle ones one of each.  For balanced
    counts this brings per-core columns from max(c_e) down to
    ~max(mid counts, (max+min)/2) -- about 100 columns (~32us) for the
    target routing.
  - Shared expert is data-parallel: core c processes tokens [c*NS,(c+1)*NS)
    with the 0.5 scale folded into Sd on host.
  - All matmul operands are bfloat16: full PE rate (1 row/cycle) like
    float32r, but LDWEIGHTS takes half the time (hidden behind >=256-row
    streams) and DMA traffic halves.  PSUM accumulation is fp32, as is
    the cross-half-block accumulation of the down-projection in SBUF.
    Measured rel-l2 of the final output ~4.7e-3 (fp8 was evaluated and
    rejected: quantizing even one operand of one matmul to e4m3 already
    gives 2.5-3.7e-2 rel-l2, over the 2e-2 budget).
  - Single pass over each group's capacity per phase: weights stream
    through SBUF once per group (2x 25MB + shared 13MB per core, well
    under the ~350GB/s * compute-time budget).
  - All DRAM tensors are host-packed per-partition-contiguous (blocks
    matching the SBUF tiles), so every DMA is 128 descriptors of 2-8KB
    runs: descriptor generation (which blocks the issuing engine ~1us
    per 1024-descriptor transfer) stops gating startup.
  - DMA ring budget at startup: sync carries the hb0 m-blocks + first x
    chunk; gpsimd (idle until hb1's down-weights) carries the second x
    chunk and the shared-phase x; the remaining x loads trail on sync
    one half-block later so hb1's gate/up weights are never stuck
    behind them.
  - Host scatter-adds per-slot outputs (weighted by the top-k softmax
    probs) and shared outputs back into [N, H].
"""

import math

import numpy as np
import ml_dtypes

import concourse.bass as bass
import concourse.mybir as mybir
import concourse.tile as tile
from concourse import bacc
from concourse.bass_utils import run_bass_kernel_spmd

F32 = mybir.dt.float32
BF16 = mybir.dt.bfloat16
SILU = mybir.ActivationFunctionType.Silu

NP_BF16 = ml_dtypes.bfloat16

N_CORES = 8
TOP_K = 2
SHARED_SCALE = 0.5
WARMUP_GROUPS = 5  # PE p-state ramp-up groups while the first DMAs land

# Set by test harnesses to collect HW timing; harmless when False.
TRACE = False
LAST = {}

_NC_CACHE = {}


def _chunks(total, taper=False):
    """Split `total` into chunks <=512, multiples of 4, every chunk >=256
    so the LDWEIGHTS of the next matmul always hides behind the current
    stream.  With taper, the first chunk is 256 (it gates the initial x
    DMA: smaller = earlier first matmul)."""
    sizes = []
    if taper and total > 1024:
        sizes.append(512)
        total -= 512
    if total == 1024:
        sizes += [512, 256, 256]
    else:
        n = max(1, math.ceil(total / 512))
        base = (total // n) // 4 * 4
        rest = [base] * n
        rest[0] += total - base * n
        assert rest[0] <= 512, (total, rest)
        sizes += rest
    out, off = [], 0
    for sz in sizes:
        out.append((off, sz))
        off += sz
    return out


def _plan(counts):
    """Pick group capacities (CA, CB) and slot assignment.

    Returns (CA, CB, a_slots, b_slots): 8 slots per group, each
    (expert, lo, hi) into that expert's token list (hi-lo <= cap,
    possibly empty)."""
    counts = [int(c) for c in counts]
    E = len(counts)
    order = sorted(range(E), key=lambda e: -counts[e])
    best = None
    for k in range(0, E // 2 + 1):
        heavy = order[:k]
        light = order[E - k:] if k else []
        a_min = max([(counts[e] + 1) // 2 for e in heavy], default=0)
        b_min = max([(counts[e] + 1) // 2 for e in light], default=0)
        m_max = max([counts[e] for e in order[k:E - k]], default=0)
        load = max(a_min + b_min, m_max, (sum(counts) + E - 1) // E)
        if best is None or load < best[0]:
            best = (load, k, a_min, b_min)
    load, k, a_min, b_min = best
    cb = max(512, (b_min + 3) // 4 * 4)
    ca = max(512, ((max(a_min, load - cb) + 3) // 4 * 4))
    a_slots, b_slots = [], []
    for e in order[:k]:
        cut = min(ca, counts[e])
        a_slots += [(e, 0, cut), (e, cut, counts[e])]
    for e in order[E - k:] if k else []:
        cut = min(cb, counts[e])
        b_slots += [(e, 0, cut), (e, cut, counts[e])]
    for e in order[k:E - k]:
        cut = min(ca, counts[e])
        a_slots.append((e, 0, cut))
        b_slots.append((e, cut, counts[e]))
    assert len(a_slots) == E and len(b_slots) == E
    assert all(hi - lo <= ca for _, lo, hi in a_slots), (ca, a_slots)
    assert all(hi - lo <= cb for _, lo, hi in b_slots), (cb, b_slots)
    return ca, cb, a_slots, b_slots


def _build(H, I, IS, CA, CB, NS):
    """Per-core SPMD program: two expert swiglu passes over CA and CB
    capacity tokens plus shared-expert swiglu over NS tokens,
    transposed-activation layout."""
    KH = H // 128
    chs_a = _chunks(CA, taper=True)
    chs_b = _chunks(CB)
    chs_s = _chunks(NS)
    nc = bacc.Bacc("TRN2", target_bir_lowering=False)

    xa = nc.dram_tensor("xa", [128, KH * CA], BF16, kind="ExternalInput")
    wga = nc.dram_tensor("wga", [128, KH * I], BF16, kind="ExternalInput")
    wua = nc.dram_tensor("wua", [128, KH * I], BF16, kind="ExternalInput")
    wda = nc.dram_tensor("wda", [128, I * H // 128], BF16, kind="ExternalInput")
    xb = nc.dram_tensor("xb", [128, KH * CB], BF16, kind="ExternalInput")
    wgb = nc.dram_tensor("wgb", [128, KH * I], BF16, kind="ExternalInput")
    wub = nc.dram_tensor("wub", [128, KH * I], BF16, kind="ExternalInput")
    wdb = nc.dram_tensor("wdb", [128, I * H // 128], BF16, kind="ExternalInput")
    xsT = nc.dram_tensor("xsT", [128, KH * NS], BF16, kind="ExternalInput")
    sg = nc.dram_tensor("sg", [128, KH * IS], BF16, kind="ExternalInput")
    su = nc.dram_tensor("su", [128, KH * IS], BF16, kind="ExternalInput")
    sd = nc.dram_tensor("sd", [128, IS * H // 128], BF16, kind="ExternalInput")
    yTa = nc.dram_tensor("yTa", [H, CA], BF16, kind="ExternalOutput")
    yTb = nc.dram_tensor("yTb", [H, CB], BF16, kind="ExternalOutput")
    ysT = nc.dram_tensor("ysT", [H, NS], BF16, kind="ExternalOutput")

    yTa_r = yTa[:, :].rearrange("(k p) c -> p k c", p=128)
    yTb_r = yTb[:, :].rearrange("(k p) c -> p k c", p=128)
    ysT_r = ysT[:, :].rearrange("(k p) c -> p k c", p=128)

    def gu_hb(t, hb):  # [128, KH, 512] slice of a packed gate/up tensor
        return t[:, hb * KH * 512 : (hb + 1) * KH * 512].rearrange(
            "p (k c) -> p k c", k=KH
        )

    def gu_hb0_m(t, m):  # hb0 of the group-A tensors is m-blocked
        return t[:, m * KH * 128 : (m + 1) * KH * 128].rearrange(
            "p (k c) -> p k c", k=KH
        )

    def d_hb(t, hb):  # [128, 4, H] slice of a packed down tensor
        return t[:, hb * 4 * H : (hb + 1) * 4 * H].rearrange(
            "p (t c) -> p t c", t=4
        )

    def x_chunk(t, base, cn, kn=KH):  # [128, kn, cn] block of packed x
        return t[:, base : base + kn * cn].rearrange("p (k c) -> p k c", k=kn)

    with tile.TileContext(nc) as tc:
        with (
            tc.tile_pool(name="xp", bufs=1) as xp,
            tc.tile_pool(name="yp", bufs=1) as yp,
            tc.tile_pool(name="wp", bufs=6) as wp,
            tc.tile_pool(name="swp", bufs=1) as swp,
            tc.tile_pool(name="hp", bufs=2) as hp,
            tc.tile_pool(name="op", bufs=10) as op,
            tc.tile_pool(name="ps", bufs=2, space="PSUM") as ps,
        ):
            # PE warm-up: dummy accumulation groups on a memset tile keep
            # the tensor engine clocking up while the first real DMAs land
            wm = op.tile([128, 256], BF16, tag="warm")
            with tc.high_priority():
                nc.gpsimd.memset(wm, 0.0)
                for _ in range(WARMUP_GROUPS):
                    pw = ps.tile([128, 256], F32, tag="pw")
                    for k in range(8):
                        nc.tensor.matmul(
                            pw, wm[:, :128], wm[:, :],
                            start=(k == 0), stop=(k == 7),
                        )

            def mlp(x_tiles, chunk_list, y_sb, g_t, u_t, d_t, i_dim,
                    y_out_r, after_w0=None, w0_split=False, preload0=None,
                    at_hb=None):
                n_hb = i_dim // 512  # half-blocks of 512 intermediate cols
                for hb in range(n_hb):
                    g0_mblock = False
                    if hb == 0 and preload0 is not None:
                        g_sb, u_sb, d_sb = preload0
                    elif hb == 0 and w0_split:
                        # m-blocked layout: each [128, KH, 128] block is
                        # one contiguous run per partition; the first
                        # matmul only waits for block 0 + the first x.
                        # Interleave across the two fast hardware rings
                        # (sync/scalar ~200GB/s; the gpsimd ring is only
                        # ~100GB/s and starts late) in consumption order.
                        g0_mblock = True
                        g_sb = wp.tile([128, 4, KH, 128], BF16, tag="w")
                        u_sb = wp.tile([128, 4, KH, 128], BF16, tag="w")
                        # Spread the four m-blocks over three rings, in
                        # consumption order: sync is busy with the first
                        # x chunk (1MB), so m0/m2 go to scalar (lands m0
                        # by ~9us), m1 to the otherwise-idle gpsimd ring,
                        # and only m3 queues on sync behind the x chunk.
                        m_eng = [nc.scalar, nc.gpsimd, nc.scalar, nc.sync]
                        for m in range(4):
                            eng = m_eng[m]
                            eng.dma_start(out=g_sb[:, m], in_=gu_hb0_m(g_t, m))
                            eng.dma_start(out=u_sb[:, m], in_=gu_hb0_m(u_t, m))
                        d_sb = wp.tile([128, 4, H], BF16, tag="w")
                        nc.scalar.dma_start(out=d_sb, in_=d_hb(d_t, 0))
                    else:
                        g_sb = wp.tile([128, KH, 512], BF16, tag="w")
                        nc.sync.dma_start(out=g_sb, in_=gu_hb(g_t, hb))
                        u_sb = wp.tile([128, KH, 512], BF16, tag="w")
                        nc.sync.dma_start(out=u_sb, in_=gu_hb(u_t, hb))
                        d_sb = wp.tile([128, 4, H], BF16, tag="w")
                        nc.gpsimd.dma_start(out=d_sb, in_=d_hb(d_t, hb))

                    def g_sl(k, m):
                        if g0_mblock:
                            return g_sb[:, m, k, :]
                        return g_sb[:, k, m * 128 : (m + 1) * 128]

                    def u_sl(k, m):
                        if g0_mblock:
                            return u_sb[:, m, k, :]
                        return u_sb[:, k, m * 128 : (m + 1) * 128]

                    if hb == 0 and after_w0 is not None:
                        after_w0()
                    if at_hb is not None and hb in at_hb:
                        at_hb[hb]()
                    for ci, (c_off, cn) in enumerate(chunk_list):
                        x_sb = x_tiles[ci]
                        h_sb = hp.tile([128, 4, cn], BF16, tag="h")
                        x_sl = [x_sb[:, k, :] for k in range(KH)]
                        for m in range(4):
                            pg = ps.tile([128, cn], F32, tag="pg")
                            for k in range(KH):
                                nc.tensor.matmul(
                                    pg, g_sl(k, m), x_sl[k],
                                    start=(k == 0), stop=(k == KH - 1),
                                )
                            nc.scalar.activation(h_sb[:, m, :], pg, SILU)
                            pu = ps.tile([128, cn], F32, tag="pu")
                            for k in range(KH):
                                nc.tensor.matmul(
                                    pu, u_sl(k, m), x_sl[k],
                                    start=(k == 0), stop=(k == KH - 1),
                                )
                            nc.vector.tensor_mul(h_sb[:, m, :], h_sb[:, m, :], pu)
                        for hm in range(KH):
                            pd = ps.tile([128, cn], F32, tag="pd")
                            for k in range(4):
                                nc.tensor.matmul(
                                    pd,
                                    d_sb[:, k, hm * 128 : (hm + 1) * 128],
                                    h_sb[:, k, :],
                                    start=(k == 0), stop=(k == 3),
                                )
                            y_sl = y_sb[:, hm, c_off : c_off + cn]
                            if hb == 0:
                                nc.vector.tensor_copy(y_sl, pd)
                            elif hb < n_hb - 1:
                                nc.vector.tensor_add(y_sl, y_sl, pd)
                            else:
                                yo = op.tile([128, cn], BF16, tag="yo")
                                nc.vector.tensor_add(yo, y_sl, pd)
                                # never the scalar ring: a DMA trigger
                                # waiting for its data blocks the engine
                                # head-of-line, and scalar must keep
                                # running silu
                                eng = nc.sync if hm % 2 == 0 else nc.gpsimd
                                eng.dma_start(
                                    out=y_out_r[:, hm, c_off : c_off + cn],
                                    in_=yo,
                                )

            # ---- x tiles for all three phases, loaded up front
            xa_tiles = [
                xp.tile([128, KH, cn], BF16, tag=f"xa{ci}", name=f"xa{ci}")
                for ci, (_, cn) in enumerate(chs_a)
            ]
            xb_tiles = [
                xp.tile([128, KH, cn], BF16, tag=f"xb{ci}", name=f"xb{ci}")
                for ci, (_, cn) in enumerate(chs_b)
            ]
            xs_tiles = [
                xp.tile([128, KH, cn], BF16, tag=f"xs{ci}", name=f"xs{ci}")
                for ci, (_, cn) in enumerate(chs_s)
            ]
            # first chunk's x: gates the first matmul
            nc.sync.dma_start(
                out=xa_tiles[0], in_=x_chunk(xa, 0, chs_a[0][1])
            )

            def after_w0():
                # group A's remaining chunks trail on sync behind the odd
                # hb0 m-blocks; with a 512-wide first chunk they land with
                # >15us of margin
                base = KH * chs_a[0][1]
                for ci in range(1, len(chs_a)):
                    cn = chs_a[ci][1]
                    nc.sync.dma_start(out=xa_tiles[ci], in_=x_chunk(xa, base, cn))
                    base += KH * cn

            def load_xs():
                # shared-phase x on the gpsimd ring, mid-pass-A: needed
                # only at the final phase, and issuing it at startup was
                # oversubscribing the ~358GB/s per-core HBM budget right
                # when the first x chunks and hb0/hb1 weights stream
                base = 0
                for ci, (_, cn) in enumerate(chs_s):
                    nc.gpsimd.dma_start(
                        out=xs_tiles[ci], in_=x_chunk(xsT, base, cn)
                    )
                    base += KH * cn

            def load_xb():
                base = 0
                for ci, (_, cn) in enumerate(chs_b):
                    nc.sync.dma_start(out=xb_tiles[ci], in_=x_chunk(xb, base, cn))
                    base += KH * cn

            y_a = yp.tile([128, KH, CA], F32, tag="y")

            # next-phase hb0 weights: dedicated tiles on the scalar ring
            # (idle after startup), prefetched with a priority that slots
            # them right after the startup DMAs — the wp pool's rotating
            # loads run just-in-time and the phase transition would stall
            # on them otherwise.  The two transitions share one tag set:
            # the shared-expert generation reuses the pass-B tiles' space
            # once pass B's hb0 has consumed them.
            sw = {}

            def prefetch_w0(gen, g_t, u_t, d_t):
                # dedicated tiles: no pool-rotation WAR wait, so normal
                # priority suffices — a priority boost would hoist these
                # 1MB transfers into the startup HBM crunch
                g = swp.tile([128, KH, 512], BF16, tag="swg")
                u = swp.tile([128, KH, 512], BF16, tag="swu")
                dd = swp.tile([128, 4, H], BF16, tag="swd")
                nc.scalar.dma_start(out=g, in_=gu_hb(g_t, 0))
                nc.scalar.dma_start(out=u, in_=gu_hb(u_t, 0))
                nc.scalar.dma_start(out=dd, in_=d_hb(d_t, 0))
                sw[gen] = (g, u, dd)

            # ---- expert pass A
            mlp(xa_tiles, chs_a, y_a, wga, wua, wda, I, yTa_r,
                after_w0=after_w0, w0_split=True,
                at_hb={1: load_xb,
                       3: lambda: prefetch_w0("b", wgb, wub, wdb),
                       4: load_xs})

            # ---- expert pass B
            y_b = yp.tile([128, KH, CB], F32, tag="y")
            mlp(xb_tiles, chs_b, y_b, wgb, wub, wdb, I, yTb_r,
                preload0=sw["b"],
                at_hb={2: lambda: prefetch_w0("s", sg, su, sd)})

            # ---- shared-expert phase: this core's 1/8 shard of all tokens
            ys_sb = yp.tile([128, KH, NS], F32, tag="y")
            mlp(xs_tiles, chs_s, ys_sb, sg, su, sd, IS, ysT_r,
                preload0=sw["s"])

    nc.compile()
    return nc


def _pack_gu(w, m_block_hb0=False):
    """[K, N] gate/up weights -> [128, K//128 * N] per-partition-contiguous
    half-block-major blocks (hb0 m-blocked when requested)."""
    K, N = w.shape
    KT = K // 128
    w4 = w.reshape(KT, 128, N // 512, 512).transpose(1, 2, 0, 3)  # p hb k j
    if m_block_hb0:
        hb0 = w4[:, 0].reshape(128, KT, 4, 128).transpose(0, 2, 1, 3)
        return np.ascontiguousarray(
            np.concatenate(
                [hb0.reshape(128, -1), w4[:, 1:].reshape(128, -1)], axis=1
            )
        )
    return np.ascontiguousarray(w4.reshape(128, -1))


def _pack_d(w):
    """[I, H] down weights -> [128, I*H//128] half-block-major blocks."""
    I_, H_ = w.shape
    w4 = w.reshape(I_ // 512, 4, 128, H_).transpose(2, 0, 1, 3)  # p hb t j
    return np.ascontiguousarray(w4.reshape(128, -1))


def _pack_x(xTf, chunks):
    """[H, C] activations -> [128, H//128 * C] chunk-major blocks."""
    H_, C_ = xTf.shape
    xk = xTf.reshape(H_ // 128, 128, C_)
    return np.concatenate(
        [
            xk[:, :, lo : lo + sz].transpose(1, 0, 2).reshape(128, -1)
            for lo, sz in chunks
        ],
        axis=1,
    )


def _install_trace_hook():
    """run_bass_kernel_spmd(trace=True) under axon needs antenv.axon_hooks,
    absent from this image; shim it from trn_agent_boot."""
    import sys
    import types

    if "antenv.axon_hooks" in sys.modules:
        return
    from trn_agent_boot.trn_boot import _ntff_profile_via_ctypes

    hook = _ntff_profile_via_ctypes("/opt/axon/libaxon_pjrt.so")
    mod = types.ModuleType("antenv.axon_hooks")
    mod.get_axon_ntff_profile_hook = lambda: hook
    sys.modules["antenv.axon_hooks"] = mod


def kernel(hidden_states, Wr, Wg, Wu, Wd, Sg, Su, Sd):
    hidden_states = np.asarray(hidden_states, dtype=np.float32)
    Wr = np.asarray(Wr, dtype=np.float32)
    Wg = np.asarray(Wg, dtype=np.float32)
    Wu = np.asarray(Wu, dtype=np.float32)
    Wd = np.asarray(Wd, dtype=np.float32)
    Sg = np.asarray(Sg, dtype=np.float32)
    Su = np.asarray(Su, dtype=np.float32)
    Sd = np.asarray(Sd, dtype=np.float32)

    B, S, H = hidden_states.shape
    E = Wr.shape[1]
    I = Wg.shape[2]
    IS = Sg.shape[1]
    N = B * S
    assert E == N_CORES and N % N_CORES == 0
    NS = N // N_CORES

    flat = hidden_states.reshape(N, H)

    # host router, fp64 (softmax is monotone: top-k by logits == by probs)
    logits = flat.astype(np.float64) @ Wr.astype(np.float64)
    lm = logits.max(axis=1, keepdims=True)
    p = np.exp(logits - lm)
    p /= p.sum(axis=1, keepdims=True)
    order = np.argsort(-logits, axis=1, kind="stable")
    top = order[:, :TOP_K]

    sel = np.zeros((N, E), dtype=bool)
    np.put_along_axis(sel, top, True, axis=1)
    idx_e = [np.flatnonzero(sel[:, e]) for e in range(E)]
    counts = [len(ix) for ix in idx_e]
    CA, CB, a_slots, b_slots = _plan(counts)
    chs_a = _chunks(CA, taper=True)
    chs_b = _chunks(CB)
    chs_s = _chunks(NS)

    flatT = np.ascontiguousarray(flat.T.astype(NP_BF16))  # [H, N] bf16
    Sd_half = (Sd * np.float32(SHARED_SCALE)).astype(NP_BF16)
    sg_p = _pack_gu(Sg.astype(NP_BF16))
    su_p = _pack_gu(Su.astype(NP_BF16))
    sd_p = _pack_d(Sd_half)

    # per-expert weight packs, cached (heavy experts appear in 2 slots)
    gu_cache = {}

    def packed_w(e, m_block):
        key = (e, m_block)
        if key not in gu_cache:
            gu_cache[key] = (
                _pack_gu(Wg[e].astype(NP_BF16), m_block_hb0=m_block),
                _pack_gu(Wu[e].astype(NP_BF16), m_block_hb0=m_block),
            )
        return gu_cache[key]

    d_cache = {}

    def packed_d(e):
        if e not in d_cache:
            d_cache[e] = _pack_d(Wd[e].astype(NP_BF16))
        return d_cache[e]

    def slot_x(slot, cap, chs):
        e, lo, hi = slot
        xT = np.zeros((H, cap), NP_BF16)
        xT[:, : hi - lo] = flatT[:, idx_e[e][lo:hi]]
        return _pack_x(xT, chs)

    in_maps = []
    for c in range(N_CORES):
        ea, eb = a_slots[c][0], b_slots[c][0]
        ga, ua = packed_w(ea, True)
        gb, ub = packed_w(eb, False)
        in_maps.append(
            {
                "xa": slot_x(a_slots[c], CA, chs_a),
                "wga": ga,
                "wua": ua,
                "wda": packed_d(ea),
                "xb": slot_x(b_slots[c], CB, chs_b),
                "wgb": gb,
                "wub": ub,
                "wdb": packed_d(eb),
                "xsT": _pack_x(flatT[:, c * NS : (c + 1) * NS], chs_s),
                "sg": sg_p,
                "su": su_p,
                "sd": sd_p,
            }
        )

    key = (H, I, IS, CA, CB, NS)
    if key not in _NC_CACHE:
        _NC_CACHE[key] = _build(*key)
    nc = _NC_CACHE[key]

    run_kwargs = {}
    if TRACE:
        _install_trace_hook()
        import tempfile

        run_kwargs = {"trace": True, "tmpdir": tempfile.mkdtemp(prefix="moe_trace_")}
    res = run_bass_kernel_spmd(nc, in_maps, core_ids=list(range(N_CORES)), **run_kwargs)
    LAST["exec_time_ns"] = res.exec_time_ns
    LAST["profile_json"] = res.profile_json
    LAST["counts"] = counts
    LAST["C"] = (CA, CB)

    out = np.zeros((N, H), np.float32)
    for c in range(N_CORES):
        for slot, yname in ((a_slots[c], "yTa"), (b_slots[c], "yTb")):
            e, lo, hi = slot
            if hi == lo:
                continue
            ix = idx_e[e][lo:hi]
            w = p[ix, e].astype(np.float32)
            out[ix] += (
                res.results[c][yname][:, : hi - lo].T.astype(np.float32)
                * w[:, None]
            )
        out[c * NS : (c + 1) * NS] += res.results[c]["ysT"].T.astype(np.float32)
    return out.reshape(B, S, H)


# revision 14
# speedup vs baseline: 1.0079x; 1.0079x over previous
"""MoE routing kernel for Trainium2 (8 NeuronCores, expert-parallel).

Strategy:
  - Router (tiny: [N,H]@[H,E]) runs on host in fp64; top-2 selection is
    identical to the fp32 reference whenever the prob gap exceeds fp32
    noise (~1e-7; measured min gap is ~6.6e-6 for the target inputs).
  - Two-group expert parallelism: SPMD forces an identical program on
    all 8 cores, so per-core capacity is uniform and the naive layout
    (expert e -> core e) pays max(counts) columns on every core.
    Instead each core runs TWO expert passes with capacities (CA, CB):
    16 slots total, each slot = a token-slice of one expert (weights are
    per-core data).  A small host solver picks (CA, CB) and the
    slot assignment: the k heaviest experts take two A-slots, the k
    lightest two B-slots, the middle ones one of each.  For balanced
    counts this brings per-core columns from max(c_e) down to
    ~max(mid counts, (max+min)/2) -- about 100 columns (~32us) for the
    target routing.
  - Shared expert is data-parallel: core c processes tokens [c*NS,(c+1)*NS)
    with the 0.5 scale folded into Sd on host.
  - All matmul operands are bfloat16: full PE rate (1 row/cycle) like
    float32r, but LDWEIGHTS takes half the time (hidden behind >=256-row
    streams) and DMA traffic halves.  PSUM accumulation is fp32, as is
    the cross-half-block accumulation of the down-projection in SBUF.
    Measured rel-l2 of the final output ~4.7e-3 (fp8 was evaluated and
    rejected: quantizing even one operand of one matmul to e4m3 already
    gives 2.5-3.7e-2 rel-l2, over the 2e-2 budget).
  - Single pass over each group's capacity per phase: weights stream
    through SBUF once per group (2x 25MB + shared 13MB per core, well
    under the ~350GB/s * compute-time budget).
  - All DRAM tensors are host-packed per-partition-contiguous (blocks
    matching the SBUF tiles), so every DMA is 128 descriptors of 2-8KB
    runs: descriptor generation (which blocks the issuing engine ~1us
    per 1024-descriptor transfer) stops gating startup.
  - DMA ring budget at startup: sync carries the hb0 m-blocks + first x
    chunk; gpsimd (idle until hb1's down-weights) carries the second x
    chunk and the shared-phase x; the remaining x loads trail on sync
    one half-block later so hb1's gate/up weights are never stuck
    behind them.
  - Host scatter-adds per-slot outputs (weighted by the top-k softmax
    probs) and shared outputs back into [N, H].
"""

import math

import numpy as np
import ml_dtypes

import concourse.bass as bass
import concourse.mybir as mybir
import concourse.tile as tile
from concourse import bacc
from concourse.bass_utils import run_bass_kernel_spmd

F32 = mybir.dt.float32
BF16 = mybir.dt.bfloat16
SILU = mybir.ActivationFunctionType.Silu

NP_BF16 = ml_dtypes.bfloat16

N_CORES = 8
TOP_K = 2
SHARED_SCALE = 0.5
WARMUP_GROUPS = 5  # PE p-state ramp-up groups while the first DMAs land

# Set by test harnesses to collect HW timing; harmless when False.
TRACE = False
LAST = {}

_NC_CACHE = {}


def _chunks(total, taper=False):
    """Split `total` into chunks <=512, multiples of 4, every chunk >=256
    so the LDWEIGHTS of the next matmul always hides behind the current
    stream.  With taper, the first chunk is 256 (it gates the initial x
    DMA: smaller = earlier first matmul)."""
    sizes = []
    if taper and total > 1024:
        sizes.append(512)
        total -= 512
    if total == 1024:
        sizes += [512, 256, 256]
    else:
        n = max(1, math.ceil(total / 512))
        base = (total // n) // 4 * 4
        rest = [base] * n
        rest[0] += total - base * n
        assert rest[0] <= 512, (total, rest)
        sizes += rest
    out, off = [], 0
    for sz in sizes:
        out.append((off, sz))
        off += sz
    return out


def _plan(counts):
    """Pick group capacities (CA, CB) and slot assignment.

    Returns (CA, CB, a_slots, b_slots): 8 slots per group, each
    (expert, lo, hi) into that expert's token list (hi-lo <= cap,
    possibly empty)."""
    counts = [int(c) for c in counts]
    E = len(counts)
    order = sorted(range(E), key=lambda e: -counts[e])
    best = None
    for k in range(0, E // 2 + 1):
        heavy = order[:k]
        light = order[E - k:] if k else []
        a_min = max([(counts[e] + 1) // 2 for e in heavy], default=0)
        b_min = max([(counts[e] + 1) // 2 for e in light], default=0)
        m_max = max([counts[e] for e in order[k:E - k]], default=0)
        load = max(a_min + b_min, m_max, (sum(counts) + E - 1) // E)
        if best is None or load < best[0]:
            best = (load, k, a_min, b_min)
    load, k, a_min, b_min = best
    cb = max(512, (b_min + 3) // 4 * 4)
    ca = max(512, ((max(a_min, load - cb) + 3) // 4 * 4))
    a_slots, b_slots = [], []
    for e in order[:k]:
        cut = min(ca, counts[e])
        a_slots += [(e, 0, cut), (e, cut, counts[e])]
    for e in order[E - k:] if k else []:
        cut = min(cb, counts[e])
        b_slots += [(e, 0, cut), (e, cut, counts[e])]
    for e in order[k:E - k]:
        cut = min(ca, counts[e])
        a_slots.append((e, 0, cut))
        b_slots.append((e, cut, counts[e]))
    assert len(a_slots) == E and len(b_slots) == E
    assert all(hi - lo <= ca for _, lo, hi in a_slots), (ca, a_slots)
    assert all(hi - lo <= cb for _, lo, hi in b_slots), (cb, b_slots)
    return ca, cb, a_slots, b_slots


def _build(H, I, IS, CA, CB, NS):
    """Per-core SPMD program: two expert swiglu passes over CA and CB
    capacity tokens plus shared-expert swiglu over NS tokens,
    transposed-activation layout."""
    KH = H // 128
    chs_a = _chunks(CA, taper=True)
    chs_b = _chunks(CB)
    chs_s = _chunks(NS)
    nc = bacc.Bacc("TRN2", target_bir_lowering=False)

    xa = nc.dram_tensor("xa", [128, KH * CA], BF16, kind="ExternalInput")
    wga = nc.dram_tensor("wga", [128, KH * I], BF16, kind="ExternalInput")
    wua = nc.dram_tensor("wua", [128, KH * I], BF16, kind="ExternalInput")
    wda = nc.dram_tensor("wda", [128, I * H // 128], BF16, kind="ExternalInput")
    xb = nc.dram_tensor("xb", [128, KH * CB], BF16, kind="ExternalInput")
    wgb = nc.dram_tensor("wgb", [128, KH * I], BF16, kind="ExternalInput")
    wub = nc.dram_tensor("wub", [128, KH * I], BF16, kind="ExternalInput")
    wdb = nc.dram_tensor("wdb", [128, I * H // 128], BF16, kind="ExternalInput")
    xsT = nc.dram_tensor("xsT", [128, KH * NS], BF16, kind="ExternalInput")
    sg = nc.dram_tensor("sg", [128, KH * IS], BF16, kind="ExternalInput")
    su = nc.dram_tensor("su", [128, KH * IS], BF16, kind="ExternalInput")
    sd = nc.dram_tensor("sd", [128, IS * H // 128], BF16, kind="ExternalInput")
    yTa = nc.dram_tensor("yTa", [H, CA], BF16, kind="ExternalOutput")
    yTb = nc.dram_tensor("yTb", [H, CB], BF16, kind="ExternalOutput")
    ysT = nc.dram_tensor("ysT", [H, NS], BF16, kind="ExternalOutput")

    yTa_r = yTa[:, :].rearrange("(k p) c -> p k c", p=128)
    yTb_r = yTb[:, :].rearrange("(k p) c -> p k c", p=128)
    ysT_r = ysT[:, :].rearrange("(k p) c -> p k c", p=128)

    def gu_hb(t, hb):  # [128, KH, 512] slice of a packed gate/up tensor
        return t[:, hb * KH * 512 : (hb + 1) * KH * 512].rearrange(
            "p (k c) -> p k c", k=KH
        )

    def gu_hb0_m(t, m):  # hb0 of the group-A tensors is m-blocked
        return t[:, m * KH * 128 : (m + 1) * KH * 128].rearrange(
            "p (k c) -> p k c", k=KH
        )

    def d_hb(t, hb):  # [128, 4, H] slice of a packed down tensor
        return t[:, hb * 4 * H : (hb + 1) * 4 * H].rearrange(
            "p (t c) -> p t c", t=4
        )

    def x_chunk(t, base, cn, kn=KH):  # [128, kn, cn] block of packed x
        return t[:, base : base + kn * cn].rearrange("p (k c) -> p k c", k=kn)

    with tile.TileContext(nc) as tc:
        with (
            tc.tile_pool(name="xp", bufs=1) as xp,
            tc.tile_pool(name="yp", bufs=1) as yp,
            tc.tile_pool(name="wp", bufs=6) as wp,
            tc.tile_pool(name="swp", bufs=1) as swp,
            tc.tile_pool(name="hp", bufs=2) as hp,
            tc.tile_pool(name="op", bufs=10) as op,
            tc.tile_pool(name="ps", bufs=2, space="PSUM") as ps,
        ):
            # PE warm-up: dummy accumulation groups on a memset tile keep
            # the tensor engine clocking up while the first real DMAs land
            wm = op.tile([128, 256], BF16, tag="warm")
            with tc.high_priority():
                nc.gpsimd.memset(wm, 0.0)
                for _ in range(WARMUP_GROUPS):
                    pw = ps.tile([128, 256], F32, tag="pw")
                    for k in range(8):
                        nc.tensor.matmul(
                            pw, wm[:, :128], wm[:, :],
                            start=(k == 0), stop=(k == 7),
                        )

            def mlp(x_tiles, chunk_list, y_sb, g_t, u_t, d_t, i_dim,
                    y_out_r, after_w0=None, w0_split=False, preload0=None,
                    at_hb=None):
                n_hb = i_dim // 512  # half-blocks of 512 intermediate cols
                for hb in range(n_hb):
                    g0_mblock = False
                    if hb == 0 and preload0 is not None:
                        g_sb, u_sb, d_sb = preload0
                    elif hb == 0 and w0_split:
                        # m-blocked layout: each [128, KH, 128] block is
                        # one contiguous run per partition; the first
                        # matmul only waits for block 0 + the first x.
                        # Interleave across the two fast hardware rings
                        # (sync/scalar ~200GB/s; the gpsimd ring is only
                        # ~100GB/s and starts late) in consumption order.
                        g0_mblock = True
                        g_sb = wp.tile([128, 4, KH, 128], BF16, tag="w")
                        u_sb = wp.tile([128, 4, KH, 128], BF16, tag="w")
                        # Spread the four m-blocks over three rings, in
                        # consumption order: sync is busy with the first
                        # x chunk (1MB), so m0/m2 go to scalar (lands m0
                        # by ~9us), m1 to the otherwise-idle gpsimd ring,
                        # and only m3 queues on sync behind the x chunk.
                        m_eng = [nc.scalar, nc.gpsimd, nc.scalar, nc.sync]
                        for m in range(4):
                            eng = m_eng[m]
                            eng.dma_start(out=g_sb[:, m], in_=gu_hb0_m(g_t, m))
                            eng.dma_start(out=u_sb[:, m], in_=gu_hb0_m(u_t, m))
                        d_sb = wp.tile([128, 4, H], BF16, tag="w")
                        nc.scalar.dma_start(out=d_sb, in_=d_hb(d_t, 0))
                    else:
                        g_sb = wp.tile([128, KH, 512], BF16, tag="w")
                        nc.sync.dma_start(out=g_sb, in_=gu_hb(g_t, hb))
                        u_sb = wp.tile([128, KH, 512], BF16, tag="w")
                        nc.sync.dma_start(out=u_sb, in_=gu_hb(u_t, hb))
                        d_sb = wp.tile([128, 4, H], BF16, tag="w")
                        nc.gpsimd.dma_start(out=d_sb, in_=d_hb(d_t, hb))

                    def g_sl(k, m):
                        if g0_mblock:
                            return g_sb[:, m, k, :]
                        return g_sb[:, k, m * 128 : (m + 1) * 128]

                    def u_sl(k, m):
                        if g0_mblock:
                            return u_sb[:, m, k, :]
                        return u_sb[:, k, m * 128 : (m + 1) * 128]

                    if hb == 0 and after_w0 is not None:
                        after_w0()
                    if at_hb is not None and hb in at_hb:
                        at_hb[hb]()
                    for ci, (c_off, cn) in enumerate(chunk_list):
                        x_sb = x_tiles[ci]
                        h_sb = hp.tile([128, 4, cn], BF16, tag="h")
                        x_sl = [x_sb[:, k, :] for k in range(KH)]
                        for m in range(4):
                            pg = ps.tile([128, cn], F32, tag="pg")
                            for k in range(KH):
                                nc.tensor.matmul(
                                    pg, g_sl(k, m), x_sl[k],
                                    start=(k == 0), stop=(k == KH - 1),
                                )
                            nc.scalar.activation(h_sb[:, m, :], pg, SILU)
                            pu = ps.tile([128, cn], F32, tag="pu")
                            for k in range(KH):
                                nc.tensor.matmul(
                                    pu, u_sl(k, m), x_sl[k],
                                    start=(k == 0), stop=(k == KH - 1),
                                )
                            nc.vector.tensor_mul(h_sb[:, m, :], h_sb[:, m, :], pu)
                        for hm in range(KH):
                            pd = ps.tile([128, cn], F32, tag="pd")
                            for k in range(4):
                                nc.tensor.matmul(
                                    pd,
                                    d_sb[:, k, hm * 128 : (hm + 1) * 128],
                                    h_sb[:, k, :],
                                    start=(k == 0), stop=(k == 3),
                                )
                            y_sl = y_sb[:, hm, c_off : c_off + cn]
                            if hb == 0:
                                nc.vector.tensor_copy(y_sl, pd)
                            elif hb < n_hb - 1:
                                nc.vector.tensor_add(y_sl, y_sl, pd)
                            else:
                                yo = op.tile([128, cn], BF16, tag="yo")
                                nc.vector.tensor_add(yo, y_sl, pd)
                                # never the scalar ring: a DMA trigger
                                # waiting for its data blocks the engine
                                # head-of-line, and scalar must keep
                                # running silu
                                eng = nc.sync if hm % 2 == 0 else nc.gpsimd
                                eng.dma_start(
                                    out=y_out_r[:, hm, c_off : c_off + cn],
                                    in_=yo,
                                )

            # ---- x tiles for all three phases, loaded up front
            xa_tiles = [
                xp.tile([128, KH, cn], BF16, tag=f"xa{ci}", name=f"xa{ci}")
                for ci, (_, cn) in enumerate(chs_a)
            ]
            xb_tiles = [
                xp.tile([128, KH, cn], BF16, tag=f"xb{ci}", name=f"xb{ci}")
                for ci, (_, cn) in enumerate(chs_b)
            ]
            xs_tiles = [
                xp.tile([128, KH, cn], BF16, tag=f"xs{ci}", name=f"xs{ci}")
                for ci, (_, cn) in enumerate(chs_s)
            ]
            # first chunk's x: gates the first matmul
            nc.sync.dma_start(
                out=xa_tiles[0], in_=x_chunk(xa, 0, chs_a[0][1])
            )

            def after_w0():
                # group A's remaining chunks trail on sync behind the odd
                # hb0 m-blocks; with a 512-wide first chunk they land with
                # >15us of margin
                base = KH * chs_a[0][1]
                for ci in range(1, len(chs_a)):
                    cn = chs_a[ci][1]
                    nc.sync.dma_start(out=xa_tiles[ci], in_=x_chunk(xa, base, cn))
                    base += KH * cn

            def load_xs():
                # shared-phase x mid-pass-A on the SYNC ring: sync's queue
                # is busy with per-hb weight loads, so these triggers
                # physically serialize to ~200us.  On an idle ring the
                # scheduler hoists them to the front and the 2.1MB lands
                # in the startup window, oversubscribing the ~358GB/s
                # per-core HBM budget right when the first x chunks and
                # hb0/hb1 weights stream (measured: chunk1 stalls ~4.6us).
                base = 0
                for ci, (_, cn) in enumerate(chs_s):
                    nc.sync.dma_start(
                        out=xs_tiles[ci], in_=x_chunk(xsT, base, cn)
                    )
                    base += KH * cn

            def load_xb():
                base = 0
                for ci, (_, cn) in enumerate(chs_b):
                    nc.sync.dma_start(out=xb_tiles[ci], in_=x_chunk(xb, base, cn))
                    base += KH * cn

            y_a = yp.tile([128, KH, CA], F32, tag="y")

            # next-phase hb0 weights: dedicated tiles on the scalar ring
            # (idle after startup), prefetched with a priority that slots
            # them right after the startup DMAs — the wp pool's rotating
            # loads run just-in-time and the phase transition would stall
            # on them otherwise.  The two transitions share one tag set:
            # the shared-expert generation reuses the pass-B tiles' space
            # once pass B's hb0 has consumed them.
            sw = {}

            def prefetch_w0(gen, g_t, u_t, d_t):
                # dedicated tiles: no pool-rotation WAR wait, so normal
                # priority suffices — a priority boost would hoist these
                # 1MB transfers into the startup HBM crunch
                g = swp.tile([128, KH, 512], BF16, tag="swg")
                u = swp.tile([128, KH, 512], BF16, tag="swu")
                dd = swp.tile([128, 4, H], BF16, tag="swd")
                nc.scalar.dma_start(out=g, in_=gu_hb(g_t, 0))
                nc.scalar.dma_start(out=u, in_=gu_hb(u_t, 0))
                nc.scalar.dma_start(out=dd, in_=d_hb(d_t, 0))
                sw[gen] = (g, u, dd)

            # ---- expert pass A
            mlp(xa_tiles, chs_a, y_a, wga, wua, wda, I, yTa_r,
                after_w0=after_w0, w0_split=True,
                at_hb={1: load_xb,
                       3: lambda: prefetch_w0("b", wgb, wub, wdb),
                       4: load_xs})

            # ---- expert pass B
            y_b = yp.tile([128, KH, CB], F32, tag="y")
            mlp(xb_tiles, chs_b, y_b, wgb, wub, wdb, I, yTb_r,
                preload0=sw["b"],
                at_hb={2: lambda: prefetch_w0("s", sg, su, sd)})

            # ---- shared-expert phase: this core's 1/8 shard of all tokens
            ys_sb = yp.tile([128, KH, NS], F32, tag="y")
            mlp(xs_tiles, chs_s, ys_sb, sg, su, sd, IS, ysT_r,
                preload0=sw["s"])

    nc.compile()
    return nc


def _pack_gu(w, m_block_hb0=False):
    """[K, N] gate/up weights -> [128, K//128 * N] per-partition-contiguous
    half-block-major blocks (hb0 m-blocked when requested)."""
    K, N = w.shape
    KT = K // 128
    w4 = w.reshape(KT, 128, N // 512, 512).transpose(1, 2, 0, 3)  # p hb k j
    if m_block_hb0:
        hb0 = w4[:, 0].reshape(128, KT, 4, 128).transpose(0, 2, 1, 3)
        return np.ascontiguousarray(
            np.concatenate(
                [hb0.reshape(128, -1), w4[:, 1:].reshape(128, -1)], axis=1
            )
        )
    return np.ascontiguousarray(w4.reshape(128, -1))


def _pack_d(w):
    """[I, H] down weights -> [128, I*H//128] half-block-major blocks."""
    I_, H_ = w.shape
    w4 = w.reshape(I_ // 512, 4, 128, H_).transpose(2, 0, 1, 3)  # p hb t j
    return np.ascontiguousarray(w4.reshape(128, -1))


def _pack_x(xTf, chunks):
    """[H, C] activations -> [128, H//128 * C] chunk-major blocks."""
    H_, C_ = xTf.shape
    xk = xTf.reshape(H_ // 128, 128, C_)
    return np.concatenate(
        [
            xk[:, :, lo : lo + sz].transpose(1, 0, 2).reshape(128, -1)
            for lo, sz in chunks
        ],
        axis=1,
    )


def _install_trace_hook():
    """run_bass_kernel_spmd(trace=True) under axon needs antenv.axon_hooks,
    absent from this image; shim it from trn_agent_boot."""
    import sys
    import types

    if "antenv.axon_hooks" in sys.modules:
        return
    from trn_agent_boot.trn_boot import _ntff_profile_via_ctypes

    hook = _ntff_profile_via_ctypes("/opt/axon/libaxon_pjrt.so")
    mod = types.ModuleType("antenv.axon_hooks")
    mod.get_axon_ntff_profile_hook = lambda: hook
    sys.modules["antenv.axon_hooks"] = mod


def kernel(hidden_states, Wr, Wg, Wu, Wd, Sg, Su, Sd):
    hidden_states = np.asarray(hidden_states, dtype=np.float32)
    Wr = np.asarray(Wr, dtype=np.float32)
    Wg = np.asarray(Wg, dtype=np.float32)
    Wu = np.asarray(Wu, dtype=np.float32)
    Wd = np.asarray(Wd, dtype=np.float32)
    Sg = np.asarray(Sg, dtype=np.float32)
    Su = np.asarray(Su, dtype=np.float32)
    Sd = np.asarray(Sd, dtype=np.float32)

    B, S, H = hidden_states.shape
    E = Wr.shape[1]
    I = Wg.shape[2]
    IS = Sg.shape[1]
    N = B * S
    assert E == N_CORES and N % N_CORES == 0
    NS = N // N_CORES

    flat = hidden_states.reshape(N, H)

    # host router, fp64 (softmax is monotone: top-k by logits == by probs)
    logits = flat.astype(np.float64) @ Wr.astype(np.float64)
    lm = logits.max(axis=1, keepdims=True)
    p = np.exp(logits - lm)
    p /= p.sum(axis=1, keepdims=True)
    order = np.argsort(-logits, axis=1, kind="stable")
    top = order[:, :TOP_K]

    sel = np.zeros((N, E), dtype=bool)
    np.put_along_axis(sel, top, True, axis=1)
    idx_e = [np.flatnonzero(sel[:, e]) for e in range(E)]
    counts = [len(ix) for ix in idx_e]
    CA, CB, a_slots, b_slots = _plan(counts)
    chs_a = _chunks(CA, taper=True)
    chs_b = _chunks(CB)
    chs_s = _chunks(NS)

    flatT = np.ascontiguousarray(flat.T.astype(NP_BF16))  # [H, N] bf16
    Sd_half = (Sd * np.float32(SHARED_SCALE)).astype(NP_BF16)
    sg_p = _pack_gu(Sg.astype(NP_BF16))
    su_p = _pack_gu(Su.astype(NP_BF16))
    sd_p = _pack_d(Sd_half)

    # per-expert weight packs, cached (heavy experts appear in 2 slots)
    gu_cache = {}

    def packed_w(e, m_block):
        key = (e, m_block)
        if key not in gu_cache:
            gu_cache[key] = (
                _pack_gu(Wg[e].astype(NP_BF16), m_block_hb0=m_block),
                _pack_gu(Wu[e].astype(NP_BF16), m_block_hb0=m_block),
            )
        return gu_cache[key]

    d_cache = {}

    def packed_d(e):
        if e not in d_cache:
            d_cache[e] = _pack_d(Wd[e].astype(NP_BF16))
        return d_cache[e]

    def slot_x(slot, cap, chs):
        e, lo, hi = slot
        xT = np.zeros((H, cap), NP_BF16)
        xT[:, : hi - lo] = flatT[:, idx_e[e][lo:hi]]
        return _pack_x(xT, chs)

    in_maps = []
    for c in range(N_CORES):
        ea, eb = a_slots[c][0], b_slots[c][0]
        ga, ua = packed_w(ea, True)
        gb, ub = packed_w(eb, False)
        in_maps.append(
            {
                "xa": slot_x(a_slots[c], CA, chs_a),
                "wga": ga,
                "wua": ua,
                "wda": packed_d(ea),
                "xb": slot_x(b_slots[c], CB, chs_b),
                "wgb": gb,
                "wub": ub,
                "wdb": packed_d(eb),
                "xsT": _pack_x(flatT[:, c * NS : (c + 1) * NS], chs_s),
                "sg": sg_p,
                "su": su_p,
                "sd": sd_p,
            }
        )

    key = (H, I, IS, CA, CB, NS)
    if key not in _NC_CACHE:
        _NC_CACHE[key] = _build(*key)
    nc = _NC_CACHE[key]

    run_kwargs = {}
    if TRACE:
        _install_trace_hook()
        import tempfile

        run_kwargs = {"trace": True, "tmpdir": tempfile.mkdtemp(prefix="moe_trace_")}
    res = run_bass_kernel_spmd(nc, in_maps, core_ids=list(range(N_CORES)), **run_kwargs)
    LAST["exec_time_ns"] = res.exec_time_ns
    LAST["profile_json"] = res.profile_json
    LAST["counts"] = counts
    LAST["C"] = (CA, CB)

    out = np.zeros((N, H), np.float32)
    for c in range(N_CORES):
        for slot, yname in ((a_slots[c], "yTa"), (b_slots[c], "yTb")):
            e, lo, hi = slot
            if hi == lo:
                continue
            ix = idx_e[e][lo:hi]
            w = p[ix, e].astype(np.float32)
            out[ix] += (
                res.results[c][yname][:, : hi - lo].T.astype(np.float32)
                * w[:, None]
            )
        out[c * NS : (c + 1) * NS] += res.results[c]["ysT"].T.astype(np.float32)
    return out.reshape(B, S, H)


# revision 15
# speedup vs baseline: 1.0092x; 1.0012x over previous
"""MoE routing kernel for Trainium2 (8 NeuronCores, expert-parallel).

Strategy:
  - Router (tiny: [N,H]@[H,E]) runs on host in fp64; top-2 selection is
    identical to the fp32 reference whenever the prob gap exceeds fp32
    noise (~1e-7; measured min gap is ~6.6e-6 for the target inputs).
  - Two-group expert parallelism: SPMD forces an identical program on
    all 8 cores, so per-core capacity is uniform and the naive layout
    (expert e -> core e) pays max(counts) columns on every core.
    Instead each core runs TWO expert passes with capacities (CA, CB):
    16 slots total, each slot = a token-slice of one expert (weights are
    per-core data).  A small host solver picks (CA, CB) and the
    slot assignment: the k heaviest experts take two A-slots, the k
    lightest two B-slots, the middle ones one of each.  For balanced
    counts this brings per-core columns from max(c_e) down to
    ~max(mid counts, (max+min)/2) -- about 100 columns (~32us) for the
    target routing.
  - Shared expert is data-parallel: core c processes tokens [c*NS,(c+1)*NS)
    with the 0.5 scale folded into Sd on host.
  - All matmul operands are bfloat16: full PE rate (1 row/cycle) like
    float32r, but LDWEIGHTS takes half the time (hidden behind >=256-row
    streams) and DMA traffic halves.  PSUM accumulation is fp32, as is
    the cross-half-block accumulation of the down-projection in SBUF.
    Measured rel-l2 of the final output ~4.7e-3 (fp8 was evaluated and
    rejected: quantizing even one operand of one matmul to e4m3 already
    gives 2.5-3.7e-2 rel-l2, over the 2e-2 budget).
  - Single pass over each group's capacity per phase: weights stream
    through SBUF once per group (2x 25MB + shared 13MB per core, well
    under the ~350GB/s * compute-time budget).
  - All DRAM tensors are host-packed per-partition-contiguous (blocks
    matching the SBUF tiles), so every DMA is 128 descriptors of 2-8KB
    runs: descriptor generation (which blocks the issuing engine ~1us
    per 1024-descriptor transfer) stops gating startup.
  - DMA ring budget at startup: sync carries the hb0 m-blocks + first x
    chunk; gpsimd (idle until hb1's down-weights) carries the second x
    chunk and the shared-phase x; the remaining x loads trail on sync
    one half-block later so hb1's gate/up weights are never stuck
    behind them.
  - Host scatter-adds per-slot outputs (weighted by the top-k softmax
    probs) and shared outputs back into [N, H].
"""

import math

import numpy as np
import ml_dtypes

import concourse.bass as bass
import concourse.mybir as mybir
import concourse.tile as tile
from concourse import bacc
from concourse.bass_utils import run_bass_kernel_spmd

F32 = mybir.dt.float32
BF16 = mybir.dt.bfloat16
SILU = mybir.ActivationFunctionType.Silu

NP_BF16 = ml_dtypes.bfloat16

N_CORES = 8
TOP_K = 2
SHARED_SCALE = 0.5
WARMUP_GROUPS = 5  # PE p-state ramp-up groups while the first DMAs land

# Set by test harnesses to collect HW timing; harmless when False.
TRACE = False
LAST = {}

_NC_CACHE = {}


def _chunks(total, taper=False):
    """Split `total` into chunks <=512, multiples of 4, every chunk >=256
    so the LDWEIGHTS of the next matmul always hides behind the current
    stream.  With taper, the first chunk is 256 (it gates the initial x
    DMA: smaller = earlier first matmul)."""
    sizes = []
    if taper and total > 1024:
        sizes.append(512)
        total -= 512
    if total == 1024:
        sizes += [512, 256, 256]
    else:
        n = max(1, math.ceil(total / 512))
        base = (total // n) // 4 * 4
        rest = [base] * n
        rest[0] += total - base * n
        assert rest[0] <= 512, (total, rest)
        sizes += rest
    out, off = [], 0
    for sz in sizes:
        out.append((off, sz))
        off += sz
    return out


def _plan(counts):
    """Pick group capacities (CA, CB) and slot assignment.

    Returns (CA, CB, a_slots, b_slots): 8 slots per group, each
    (expert, lo, hi) into that expert's token list (hi-lo <= cap,
    possibly empty)."""
    counts = [int(c) for c in counts]
    E = len(counts)
    order = sorted(range(E), key=lambda e: -counts[e])
    best = None
    for k in range(0, E // 2 + 1):
        heavy = order[:k]
        light = order[E - k:] if k else []
        a_min = max([(counts[e] + 1) // 2 for e in heavy], default=0)
        b_min = max([(counts[e] + 1) // 2 for e in light], default=0)
        m_max = max([counts[e] for e in order[k:E - k]], default=0)
        load = max(a_min + b_min, m_max, (sum(counts) + E - 1) // E)
        if best is None or load < best[0]:
            best = (load, k, a_min, b_min)
    load, k, a_min, b_min = best
    cb = max(512, (b_min + 3) // 4 * 4)
    ca = max(512, ((max(a_min, load - cb) + 3) // 4 * 4))
    a_slots, b_slots = [], []
    for e in order[:k]:
        cut = min(ca, counts[e])
        a_slots += [(e, 0, cut), (e, cut, counts[e])]
    for e in order[E - k:] if k else []:
        cut = min(cb, counts[e])
        b_slots += [(e, 0, cut), (e, cut, counts[e])]
    for e in order[k:E - k]:
        cut = min(ca, counts[e])
        a_slots.append((e, 0, cut))
        b_slots.append((e, cut, counts[e]))
    assert len(a_slots) == E and len(b_slots) == E
    assert all(hi - lo <= ca for _, lo, hi in a_slots), (ca, a_slots)
    assert all(hi - lo <= cb for _, lo, hi in b_slots), (cb, b_slots)
    return ca, cb, a_slots, b_slots


def _build(H, I, IS, CA, CB, NS):
    """Per-core SPMD program: two expert swiglu passes over CA and CB
    capacity tokens plus shared-expert swiglu over NS tokens,
    transposed-activation layout."""
    KH = H // 128
    chs_a = _chunks(CA, taper=True)
    chs_b = _chunks(CB)
    chs_s = _chunks(NS)
    nc = bacc.Bacc("TRN2", target_bir_lowering=False)

    xa = nc.dram_tensor("xa", [128, KH * CA], BF16, kind="ExternalInput")
    wga = nc.dram_tensor("wga", [128, KH * I], BF16, kind="ExternalInput")
    wua = nc.dram_tensor("wua", [128, KH * I], BF16, kind="ExternalInput")
    wda = nc.dram_tensor("wda", [128, I * H // 128], BF16, kind="ExternalInput")
    xb = nc.dram_tensor("xb", [128, KH * CB], BF16, kind="ExternalInput")
    wgb = nc.dram_tensor("wgb", [128, KH * I], BF16, kind="ExternalInput")
    wub = nc.dram_tensor("wub", [128, KH * I], BF16, kind="ExternalInput")
    wdb = nc.dram_tensor("wdb", [128, I * H // 128], BF16, kind="ExternalInput")
    xsT = nc.dram_tensor("xsT", [128, KH * NS], BF16, kind="ExternalInput")
    sg = nc.dram_tensor("sg", [128, KH * IS], BF16, kind="ExternalInput")
    su = nc.dram_tensor("su", [128, KH * IS], BF16, kind="ExternalInput")
    sd = nc.dram_tensor("sd", [128, IS * H // 128], BF16, kind="ExternalInput")
    yTa = nc.dram_tensor("yTa", [H, CA], BF16, kind="ExternalOutput")
    yTb = nc.dram_tensor("yTb", [H, CB], BF16, kind="ExternalOutput")
    ysT = nc.dram_tensor("ysT", [H, NS], BF16, kind="ExternalOutput")

    yTa_r = yTa[:, :].rearrange("(k p) c -> p k c", p=128)
    yTb_r = yTb[:, :].rearrange("(k p) c -> p k c", p=128)
    ysT_r = ysT[:, :].rearrange("(k p) c -> p k c", p=128)

    def gu_hb(t, hb):  # [128, KH, 512] slice of a packed gate/up tensor
        return t[:, hb * KH * 512 : (hb + 1) * KH * 512].rearrange(
            "p (k c) -> p k c", k=KH
        )

    def gu_hb0_m(t, m):  # hb0 of the group-A tensors is m-blocked
        return t[:, m * KH * 128 : (m + 1) * KH * 128].rearrange(
            "p (k c) -> p k c", k=KH
        )

    def d_hb(t, hb):  # [128, 4, H] slice of a packed down tensor
        return t[:, hb * 4 * H : (hb + 1) * 4 * H].rearrange(
            "p (t c) -> p t c", t=4
        )

    def x_chunk(t, base, cn, kn=KH):  # [128, kn, cn] block of packed x
        return t[:, base : base + kn * cn].rearrange("p (k c) -> p k c", k=kn)

    with tile.TileContext(nc) as tc:
        with (
            tc.tile_pool(name="xp", bufs=1) as xp,
            tc.tile_pool(name="yp", bufs=1) as yp,
            tc.tile_pool(name="wp", bufs=6) as wp,
            tc.tile_pool(name="swp", bufs=1) as swp,
            tc.tile_pool(name="hp", bufs=2) as hp,
            tc.tile_pool(name="op", bufs=10) as op,
            tc.tile_pool(name="ps", bufs=2, space="PSUM") as ps,
        ):
            # PE warm-up: dummy accumulation groups on a memset tile keep
            # the tensor engine clocking up while the first real DMAs land
            wm = op.tile([128, 256], BF16, tag="warm")
            with tc.high_priority():
                nc.gpsimd.memset(wm, 0.0)
                for _ in range(WARMUP_GROUPS):
                    pw = ps.tile([128, 256], F32, tag="pw")
                    for k in range(8):
                        nc.tensor.matmul(
                            pw, wm[:, :128], wm[:, :],
                            start=(k == 0), stop=(k == 7),
                        )

            def mlp(x_tiles, chunk_list, y_sb, g_t, u_t, d_t, i_dim,
                    y_out_r, after_w0=None, w0_split=False, preload0=None,
                    at_hb=None):
                n_hb = i_dim // 512  # half-blocks of 512 intermediate cols
                for hb in range(n_hb):
                    g0_mblock = False
                    if hb == 0 and preload0 is not None:
                        g_sb, u_sb, d_sb = preload0
                    elif hb == 0 and w0_split:
                        # m-blocked layout: each [128, KH, 128] block is
                        # one contiguous run per partition; the first
                        # matmul only waits for block 0 + the first x.
                        # Interleave across the two fast hardware rings
                        # (sync/scalar ~200GB/s; the gpsimd ring is only
                        # ~100GB/s and starts late) in consumption order.
                        g0_mblock = True
                        g_sb = wp.tile([128, 4, KH, 128], BF16, tag="w")
                        u_sb = wp.tile([128, 4, KH, 128], BF16, tag="w")
                        # Spread the four m-blocks over three rings, in
                        # consumption order: sync is busy with the first
                        # x chunk (1MB), so m0/m2 go to scalar (lands m0
                        # by ~9us), m1 to the otherwise-idle gpsimd ring,
                        # and only m3 queues on sync behind the x chunk.
                        m_eng = [nc.scalar, nc.gpsimd, nc.scalar, nc.sync]
                        for m in range(4):
                            eng = m_eng[m]
                            eng.dma_start(out=g_sb[:, m], in_=gu_hb0_m(g_t, m))
                            eng.dma_start(out=u_sb[:, m], in_=gu_hb0_m(u_t, m))
                        d_sb = wp.tile([128, 4, H], BF16, tag="w")
                        nc.scalar.dma_start(out=d_sb, in_=d_hb(d_t, 0))
                    else:
                        g_sb = wp.tile([128, KH, 512], BF16, tag="w")
                        nc.sync.dma_start(out=g_sb, in_=gu_hb(g_t, hb))
                        u_sb = wp.tile([128, KH, 512], BF16, tag="w")
                        nc.sync.dma_start(out=u_sb, in_=gu_hb(u_t, hb))
                        d_sb = wp.tile([128, 4, H], BF16, tag="w")
                        nc.gpsimd.dma_start(out=d_sb, in_=d_hb(d_t, hb))

                    def g_sl(k, m):
                        if g0_mblock:
                            return g_sb[:, m, k, :]
                        return g_sb[:, k, m * 128 : (m + 1) * 128]

                    def u_sl(k, m):
                        if g0_mblock:
                            return u_sb[:, m, k, :]
                        return u_sb[:, k, m * 128 : (m + 1) * 128]

                    if hb == 0 and after_w0 is not None:
                        after_w0()
                    if at_hb is not None and hb in at_hb:
                        at_hb[hb]()
                    for ci, (c_off, cn) in enumerate(chunk_list):
                        x_sb = x_tiles[ci]
                        h_sb = hp.tile([128, 4, cn], BF16, tag="h")
                        x_sl = [x_sb[:, k, :] for k in range(KH)]
                        for m in range(4):
                            pg = ps.tile([128, cn], F32, tag="pg")
                            for k in range(KH):
                                nc.tensor.matmul(
                                    pg, g_sl(k, m), x_sl[k],
                                    start=(k == 0), stop=(k == KH - 1),
                                )
                            nc.scalar.activation(h_sb[:, m, :], pg, SILU)
                            pu = ps.tile([128, cn], F32, tag="pu")
                            for k in range(KH):
                                nc.tensor.matmul(
                                    pu, u_sl(k, m), x_sl[k],
                                    start=(k == 0), stop=(k == KH - 1),
                                )
                            nc.vector.tensor_mul(h_sb[:, m, :], h_sb[:, m, :], pu)
                        for hm in range(KH):
                            pd = ps.tile([128, cn], F32, tag="pd")
                            for k in range(4):
                                nc.tensor.matmul(
                                    pd,
                                    d_sb[:, k, hm * 128 : (hm + 1) * 128],
                                    h_sb[:, k, :],
                                    start=(k == 0), stop=(k == 3),
                                )
                            y_sl = y_sb[:, hm, c_off : c_off + cn]
                            if hb == 0:
                                nc.vector.tensor_copy(y_sl, pd)
                            elif hb < n_hb - 1:
                                nc.vector.tensor_add(y_sl, y_sl, pd)
                            else:
                                yo = op.tile([128, cn], BF16, tag="yo")
                                nc.vector.tensor_add(yo, y_sl, pd)
                                # never the scalar ring: a DMA trigger
                                # waiting for its data blocks the engine
                                # head-of-line, and scalar must keep
                                # running silu
                                eng = nc.sync if hm % 2 == 0 else nc.gpsimd
                                eng.dma_start(
                                    out=y_out_r[:, hm, c_off : c_off + cn],
                                    in_=yo,
                                )

            # ---- x tiles for all three phases, loaded up front
            xa_tiles = [
                xp.tile([128, KH, cn], BF16, tag=f"xa{ci}", name=f"xa{ci}")
                for ci, (_, cn) in enumerate(chs_a)
            ]
            xb_tiles = [
                xp.tile([128, KH, cn], BF16, tag=f"xb{ci}", name=f"xb{ci}")
                for ci, (_, cn) in enumerate(chs_b)
            ]
            xs_tiles = [
                xp.tile([128, KH, cn], BF16, tag=f"xs{ci}", name=f"xs{ci}")
                for ci, (_, cn) in enumerate(chs_s)
            ]
            # first chunk's x: gates the first matmul
            nc.sync.dma_start(
                out=xa_tiles[0], in_=x_chunk(xa, 0, chs_a[0][1])
            )

            def after_w0():
                # group A's remaining chunks trail on sync behind the odd
                # hb0 m-blocks; with a 512-wide first chunk they land with
                # >15us of margin
                base = KH * chs_a[0][1]
                for ci in range(1, len(chs_a)):
                    cn = chs_a[ci][1]
                    nc.sync.dma_start(out=xa_tiles[ci], in_=x_chunk(xa, base, cn))
                    base += KH * cn

            def load_xs():
                # shared-phase x mid-pass-A on the SYNC ring: sync's queue
                # is busy with per-hb weight loads, so these triggers
                # physically serialize to ~200us.  On an idle ring the
                # scheduler hoists them to the front and the 2.1MB lands
                # in the startup window, oversubscribing the ~358GB/s
                # per-core HBM budget right when the first x chunks and
                # hb0/hb1 weights stream (measured: chunk1 stalls ~4.6us).
                base = 0
                for ci, (_, cn) in enumerate(chs_s):
                    nc.sync.dma_start(
                        out=xs_tiles[ci], in_=x_chunk(xsT, base, cn)
                    )
                    base += KH * cn

            def load_xb():
                base = 0
                for ci, (_, cn) in enumerate(chs_b):
                    nc.sync.dma_start(out=xb_tiles[ci], in_=x_chunk(xb, base, cn))
                    base += KH * cn

            y_a = yp.tile([128, KH, CA], F32, tag="y")

            # next-phase hb0 weights: dedicated tiles on the scalar ring
            # (idle after startup), prefetched with a priority that slots
            # them right after the startup DMAs — the wp pool's rotating
            # loads run just-in-time and the phase transition would stall
            # on them otherwise.  The two transitions share one tag set:
            # the shared-expert generation reuses the pass-B tiles' space
            # once pass B's hb0 has consumed them.
            sw = {}

            def prefetch_w0(gen, g_t, u_t, d_t):
                # dedicated tiles: no pool-rotation WAR wait.  On the SYNC
                # ring (not scalar): an idle engine queue lets the
                # scheduler run these 1MB transfers during the startup HBM
                # crunch (scalar's only other work is activations, and the
                # triggers hoisted to ~14us, delaying d0); sync's steady
                # per-hb weight traffic serializes them to mid-pass.
                g = swp.tile([128, KH, 512], BF16, tag="swg")
                u = swp.tile([128, KH, 512], BF16, tag="swu")
                dd = swp.tile([128, 4, H], BF16, tag="swd")
                nc.sync.dma_start(out=g, in_=gu_hb(g_t, 0))
                nc.sync.dma_start(out=u, in_=gu_hb(u_t, 0))
                nc.sync.dma_start(out=dd, in_=d_hb(d_t, 0))
                sw[gen] = (g, u, dd)

            # ---- expert pass A
            mlp(xa_tiles, chs_a, y_a, wga, wua, wda, I, yTa_r,
                after_w0=after_w0, w0_split=True,
                at_hb={1: load_xb,
                       3: lambda: prefetch_w0("b", wgb, wub, wdb),
                       4: load_xs})

            # ---- expert pass B
            y_b = yp.tile([128, KH, CB], F32, tag="y")
            mlp(xb_tiles, chs_b, y_b, wgb, wub, wdb, I, yTb_r,
                preload0=sw["b"],
                at_hb={2: lambda: prefetch_w0("s", sg, su, sd)})

            # ---- shared-expert phase: this core's 1/8 shard of all tokens
            ys_sb = yp.tile([128, KH, NS], F32, tag="y")
            mlp(xs_tiles, chs_s, ys_sb, sg, su, sd, IS, ysT_r,
                preload0=sw["s"])

    nc.compile()
    return nc


def _pack_gu(w, m_block_hb0=False):
    """[K, N] gate/up weights -> [128, K//128 * N] per-partition-contiguous
    half-block-major blocks (hb0 m-blocked when requested)."""
    K, N = w.shape
    KT = K // 128
    w4 = w.reshape(KT, 128, N // 512, 512).transpose(1, 2, 0, 3)  # p hb k j
    if m_block_hb0:
        hb0 = w4[:, 0].reshape(128, KT, 4, 128).transpose(0, 2, 1, 3)
        return np.ascontiguousarray(
            np.concatenate(
                [hb0.reshape(128, -1), w4[:, 1:].reshape(128, -1)], axis=1
            )
        )
    return np.ascontiguousarray(w4.reshape(128, -1))


def _pack_d(w):
    """[I, H] down weights -> [128, I*H//128] half-block-major blocks."""
    I_, H_ = w.shape
    w4 = w.reshape(I_ // 512, 4, 128, H_).transpose(2, 0, 1, 3)  # p hb t j
    return np.ascontiguousarray(w4.reshape(128, -1))


def _pack_x(xTf, chunks):
    """[H, C] activations -> [128, H//128 * C] chunk-major blocks."""
    H_, C_ = xTf.shape
    xk = xTf.reshape(H_ // 128, 128, C_)
    return np.concatenate(
        [
            xk[:, :, lo : lo + sz].transpose(1, 0, 2).reshape(128, -1)
            for lo, sz in chunks
        ],
        axis=1,
    )


def _install_trace_hook():
    """run_bass_kernel_spmd(trace=True) under axon needs antenv.axon_hooks,
    absent from this image; shim it from trn_agent_boot."""
    import sys
    import types

    if "antenv.axon_hooks" in sys.modules:
        return
    from trn_agent_boot.trn_boot import _ntff_profile_via_ctypes

    hook = _ntff_profile_via_ctypes("/opt/axon/libaxon_pjrt.so")
    mod = types.ModuleType("antenv.axon_hooks")
    mod.get_axon_ntff_profile_hook = lambda: hook
    sys.modules["antenv.axon_hooks"] = mod


def kernel(hidden_states, Wr, Wg, Wu, Wd, Sg, Su, Sd):
    hidden_states = np.asarray(hidden_states, dtype=np.float32)
    Wr = np.asarray(Wr, dtype=np.float32)
    Wg = np.asarray(Wg, dtype=np.float32)
    Wu = np.asarray(Wu, dtype=np.float32)
    Wd = np.asarray(Wd, dtype=np.float32)
    Sg = np.asarray(Sg, dtype=np.float32)
    Su = np.asarray(Su, dtype=np.float32)
    Sd = np.asarray(Sd, dtype=np.float32)

    B, S, H = hidden_states.shape
    E = Wr.shape[1]
    I = Wg.shape[2]
    IS = Sg.shape[1]
    N = B * S
    assert E == N_CORES and N % N_CORES == 0
    NS = N // N_CORES

    flat = hidden_states.reshape(N, H)

    # host router, fp64 (softmax is monotone: top-k by logits == by probs)
    logits = flat.astype(np.float64) @ Wr.astype(np.float64)
    lm = logits.max(axis=1, keepdims=True)
    p = np.exp(logits - lm)
    p /= p.sum(axis=1, keepdims=True)
    order = np.argsort(-logits, axis=1, kind="stable")
    top = order[:, :TOP_K]

    sel = np.zeros((N, E), dtype=bool)
    np.put_along_axis(sel, top, True, axis=1)
    idx_e = [np.flatnonzero(sel[:, e]) for e in range(E)]
    counts = [len(ix) for ix in idx_e]
    CA, CB, a_slots, b_slots = _plan(counts)
    chs_a = _chunks(CA, taper=True)
    chs_b = _chunks(CB)
    chs_s = _chunks(NS)

    flatT = np.ascontiguousarray(flat.T.astype(NP_BF16))  # [H, N] bf16
    Sd_half = (Sd * np.float32(SHARED_SCALE)).astype(NP_BF16)
    sg_p = _pack_gu(Sg.astype(NP_BF16))
    su_p = _pack_gu(Su.astype(NP_BF16))
    sd_p = _pack_d(Sd_half)

    # per-expert weight packs, cached (heavy experts appear in 2 slots)
    gu_cache = {}

    def packed_w(e, m_block):
        key = (e, m_block)
        if key not in gu_cache:
            gu_cache[key] = (
                _pack_gu(Wg[e].astype(NP_BF16), m_block_hb0=m_block),
                _pack_gu(Wu[e].astype(NP_BF16), m_block_hb0=m_block),
            )
        return gu_cache[key]

    d_cache = {}

    def packed_d(e):
        if e not in d_cache:
            d_cache[e] = _pack_d(Wd[e].astype(NP_BF16))
        return d_cache[e]

    def slot_x(slot, cap, chs):
        e, lo, hi = slot
        xT = np.zeros((H, cap), NP_BF16)
        xT[:, : hi - lo] = flatT[:, idx_e[e][lo:hi]]
        return _pack_x(xT, chs)

    in_maps = []
    for c in range(N_CORES):
        ea, eb = a_slots[c][0], b_slots[c][0]
        ga, ua = packed_w(ea, True)
        gb, ub = packed_w(eb, False)
        in_maps.append(
            {
                "xa": slot_x(a_slots[c], CA, chs_a),
                "wga": ga,
                "wua": ua,
                "wda": packed_d(ea),
                "xb": slot_x(b_slots[c], CB, chs_b),
                "wgb": gb,
                "wub": ub,
                "wdb": packed_d(eb),
                "xsT": _pack_x(flatT[:, c * NS : (c + 1) * NS], chs_s),
                "sg": sg_p,
                "su": su_p,
                "sd": sd_p,
            }
        )

    key = (H, I, IS, CA, CB, NS)
    if key not in _NC_CACHE:
        _NC_CACHE[key] = _build(*key)
    nc = _NC_CACHE[key]

    run_kwargs = {}
    if TRACE:
        _install_trace_hook()
        import tempfile

        run_kwargs = {"trace": True, "tmpdir": tempfile.mkdtemp(prefix="moe_trace_")}
    res = run_bass_kernel_spmd(nc, in_maps, core_ids=list(range(N_CORES)), **run_kwargs)
    LAST["exec_time_ns"] = res.exec_time_ns
    LAST["profile_json"] = res.profile_json
    LAST["counts"] = counts
    LAST["C"] = (CA, CB)

    out = np.zeros((N, H), np.float32)
    for c in range(N_CORES):
        for slot, yname in ((a_slots[c], "yTa"), (b_slots[c], "yTb")):
            e, lo, hi = slot
            if hi == lo:
                continue
            ix = idx_e[e][lo:hi]
            w = p[ix, e].astype(np.float32)
            out[ix] += (
                res.results[c][yname][:, : hi - lo].T.astype(np.float32)
                * w[:, None]
            )
        out[c * NS : (c + 1) * NS] += res.results[c]["ysT"].T.astype(np.float32)
    return out.reshape(B, S, H)


# revision 18
# speedup vs baseline: 1.0119x; 1.0027x over previous
"""MoE routing kernel for Trainium2 (8 NeuronCores, expert-parallel).

Strategy:
  - Router (tiny: [N,H]@[H,E]) runs on host in fp64; top-2 selection is
    identical to the fp32 reference whenever the prob gap exceeds fp32
    noise (~1e-7; measured min gap is ~6.6e-6 for the target inputs).
  - Two-group expert parallelism: SPMD forces an identical program on
    all 8 cores, so per-core capacity is uniform and the naive layout
    (expert e -> core e) pays max(counts) columns on every core.
    Instead each core runs TWO expert passes with capacities (CA, CB):
    16 slots total, each slot = a token-slice of one expert (weights are
    per-core data).  A small host solver picks (CA, CB) and the
    slot assignment: the k heaviest experts take two A-slots, the k
    lightest two B-slots, the middle ones one of each.  For balanced
    counts this brings per-core columns from max(c_e) down to
    ~max(mid counts, (max+min)/2) -- about 100 columns (~32us) for the
    target routing.
  - Shared expert is data-parallel: core c processes tokens [c*NS,(c+1)*NS)
    with the 0.5 scale folded into Sd on host.
  - All matmul operands are bfloat16: full PE rate (1 row/cycle) like
    float32r, but LDWEIGHTS takes half the time (hidden behind >=256-row
    streams) and DMA traffic halves.  PSUM accumulation is fp32, as is
    the cross-half-block accumulation of the down-projection in SBUF.
    Measured rel-l2 of the final output ~4.7e-3 (fp8 was evaluated and
    rejected: quantizing even one operand of one matmul to e4m3 already
    gives 2.5-3.7e-2 rel-l2, over the 2e-2 budget).
  - Single pass over each group's capacity per phase: weights stream
    through SBUF once per group (2x 25MB + shared 13MB per core, well
    under the ~350GB/s * compute-time budget).
  - All DRAM tensors are host-packed per-partition-contiguous (blocks
    matching the SBUF tiles), so every DMA is 128 descriptors of 2-8KB
    runs: descriptor generation (which blocks the issuing engine ~1us
    per 1024-descriptor transfer) stops gating startup.
  - DMA ring budget at startup: sync carries the hb0 m-blocks + first x
    chunk; gpsimd (idle until hb1's down-weights) carries the second x
    chunk and the shared-phase x; the remaining x loads trail on sync
    one half-block later so hb1's gate/up weights are never stuck
    behind them.
  - Host scatter-adds per-slot outputs (weighted by the top-k softmax
    probs) and shared outputs back into [N, H].
"""

import math

import numpy as np
import ml_dtypes

import concourse.bass as bass
import concourse.mybir as mybir
import concourse.tile as tile
from concourse import bacc
from concourse.bass_utils import run_bass_kernel_spmd

F32 = mybir.dt.float32
BF16 = mybir.dt.bfloat16
SILU = mybir.ActivationFunctionType.Silu

NP_BF16 = ml_dtypes.bfloat16

N_CORES = 8
TOP_K = 2
SHARED_SCALE = 0.5
WARMUP_GROUPS = 5  # PE p-state ramp-up groups while the first DMAs land

# Set by test harnesses to collect HW timing; harmless when False.
TRACE = False
LAST = {}

_NC_CACHE = {}


def _chunks(total, taper=False):
    """Split `total` into chunks <=512, multiples of 4, every chunk >=256
    so the LDWEIGHTS of the next matmul always hides behind the current
    stream.  With taper, the first chunk is 256 (it gates the initial x
    DMA: smaller = earlier first matmul)."""
    sizes = []
    if taper and total > 1024:
        sizes.append(512)
        total -= 512
    if total == 1024:
        sizes += [512, 256, 256]
    else:
        n = max(1, math.ceil(total / 512))
        base = (total // n) // 4 * 4
        rest = [base] * n
        rest[0] += total - base * n
        assert rest[0] <= 512, (total, rest)
        sizes += rest
    out, off = [], 0
    for sz in sizes:
        out.append((off, sz))
        off += sz
    return out


def _plan(counts):
    """Pick group capacities (CA, CB) and slot assignment.

    Returns (CA, CB, a_slots, b_slots): 8 slots per group, each
    (expert, lo, hi) into that expert's token list (hi-lo <= cap,
    possibly empty)."""
    counts = [int(c) for c in counts]
    E = len(counts)
    order = sorted(range(E), key=lambda e: -counts[e])
    best = None
    for k in range(0, E // 2 + 1):
        heavy = order[:k]
        light = order[E - k:] if k else []
        a_min = max([(counts[e] + 1) // 2 for e in heavy], default=0)
        b_min = max([(counts[e] + 1) // 2 for e in light], default=0)
        m_max = max([counts[e] for e in order[k:E - k]], default=0)
        load = max(a_min + b_min, m_max, (sum(counts) + E - 1) // E)
        if best is None or load < best[0]:
            best = (load, k, a_min, b_min)
    load, k, a_min, b_min = best
    cb = max(512, (b_min + 3) // 4 * 4)
    ca = max(512, ((max(a_min, load - cb) + 3) // 4 * 4))
    a_slots, b_slots = [], []
    for e in order[:k]:
        cut = min(ca, counts[e])
        a_slots += [(e, 0, cut), (e, cut, counts[e])]
    for e in order[E - k:] if k else []:
        cut = min(cb, counts[e])
        b_slots += [(e, 0, cut), (e, cut, counts[e])]
    for e in order[k:E - k]:
        cut = min(ca, counts[e])
        a_slots.append((e, 0, cut))
        b_slots.append((e, cut, counts[e]))
    assert len(a_slots) == E and len(b_slots) == E
    assert all(hi - lo <= ca for _, lo, hi in a_slots), (ca, a_slots)
    assert all(hi - lo <= cb for _, lo, hi in b_slots), (cb, b_slots)
    return ca, cb, a_slots, b_slots


def _build(H, I, IS, CA, CB, NS):
    """Per-core SPMD program: two expert swiglu passes over CA and CB
    capacity tokens plus shared-expert swiglu over NS tokens,
    transposed-activation layout."""
    KH = H // 128
    chs_a = _chunks(CA, taper=True)
    chs_b = _chunks(CB)
    chs_s = _chunks(NS)
    nc = bacc.Bacc("TRN2", target_bir_lowering=False)

    xa = nc.dram_tensor("xa", [128, KH * CA], BF16, kind="ExternalInput")
    wga = nc.dram_tensor("wga", [128, KH * I], BF16, kind="ExternalInput")
    wua = nc.dram_tensor("wua", [128, KH * I], BF16, kind="ExternalInput")
    wda = nc.dram_tensor("wda", [128, I * H // 128], BF16, kind="ExternalInput")
    xb = nc.dram_tensor("xb", [128, KH * CB], BF16, kind="ExternalInput")
    wgb = nc.dram_tensor("wgb", [128, KH * I], BF16, kind="ExternalInput")
    wub = nc.dram_tensor("wub", [128, KH * I], BF16, kind="ExternalInput")
    wdb = nc.dram_tensor("wdb", [128, I * H // 128], BF16, kind="ExternalInput")
    xsT = nc.dram_tensor("xsT", [128, KH * NS], BF16, kind="ExternalInput")
    sg = nc.dram_tensor("sg", [128, KH * IS], BF16, kind="ExternalInput")
    su = nc.dram_tensor("su", [128, KH * IS], BF16, kind="ExternalInput")
    sd = nc.dram_tensor("sd", [128, IS * H // 128], BF16, kind="ExternalInput")
    yTa = nc.dram_tensor("yTa", [H, CA], BF16, kind="ExternalOutput")
    yTb = nc.dram_tensor("yTb", [H, CB], BF16, kind="ExternalOutput")
    ysT = nc.dram_tensor("ysT", [H, NS], BF16, kind="ExternalOutput")

    yTa_r = yTa[:, :].rearrange("(k p) c -> p k c", p=128)
    yTb_r = yTb[:, :].rearrange("(k p) c -> p k c", p=128)
    ysT_r = ysT[:, :].rearrange("(k p) c -> p k c", p=128)

    def gu_hb(t, hb):  # [128, KH, 512] slice of a packed gate/up tensor
        return t[:, hb * KH * 512 : (hb + 1) * KH * 512].rearrange(
            "p (k c) -> p k c", k=KH
        )

    def gu_hb0_m(t, m):  # hb0 of the group-A tensors is m-blocked
        return t[:, m * KH * 128 : (m + 1) * KH * 128].rearrange(
            "p (k c) -> p k c", k=KH
        )

    def d_hb(t, hb):  # [128, 4, H] slice of a packed down tensor
        return t[:, hb * 4 * H : (hb + 1) * 4 * H].rearrange(
            "p (t c) -> p t c", t=4
        )

    def x_chunk(t, base, cn, kn=KH):  # [128, kn, cn] block of packed x
        return t[:, base : base + kn * cn].rearrange("p (k c) -> p k c", k=kn)

    with tile.TileContext(nc) as tc:
        with (
            tc.tile_pool(name="xp", bufs=1) as xp,
            tc.tile_pool(name="yp", bufs=1) as yp,
            tc.tile_pool(name="wp", bufs=6) as wp,
            tc.tile_pool(name="swp", bufs=1) as swp,
            tc.tile_pool(name="hp", bufs=2) as hp,
            tc.tile_pool(name="op", bufs=10) as op,
            tc.tile_pool(name="ps", bufs=2, space="PSUM") as ps,
        ):
            # PE warm-up: dummy accumulation groups on a memset tile keep
            # the tensor engine clocking up while the first real DMAs land
            wm = op.tile([128, 256], BF16, tag="warm")
            with tc.high_priority():
                nc.gpsimd.memset(wm, 0.0)
                for _ in range(WARMUP_GROUPS):
                    pw = ps.tile([128, 256], F32, tag="pw")
                    for k in range(8):
                        nc.tensor.matmul(
                            pw, wm[:, :128], wm[:, :],
                            start=(k == 0), stop=(k == 7),
                        )

            def mlp(x_tiles, chunk_list, y_sb, g_t, u_t, d_t, i_dim,
                    y_out_r, after_w0=None, w0_split=False, preload0=None,
                    at_hb=None):
                n_hb = i_dim // 512  # half-blocks of 512 intermediate cols
                for hb in range(n_hb):
                    g0_mblock = False
                    if hb == 0 and preload0 is not None:
                        g_sb, u_sb, d_sb = preload0
                    elif hb == 0 and w0_split:
                        # m-blocked layout: each [128, KH, 128] block is
                        # one contiguous run per partition; the first
                        # matmul only waits for block 0 + the first x.
                        # Interleave across the two fast hardware rings
                        # (sync/scalar ~200GB/s; the gpsimd ring is only
                        # ~100GB/s and starts late) in consumption order.
                        g0_mblock = True
                        g_sb = wp.tile([128, 4, KH, 128], BF16, tag="w")
                        u_sb = wp.tile([128, 4, KH, 128], BF16, tag="w")
                        # Spread the four m-blocks over three rings, in
                        # consumption order: sync is busy with the first
                        # x chunk (1MB), so m0/m2 go to scalar (lands m0
                        # by ~9us), m1 to the otherwise-idle gpsimd ring,
                        # and only m3 queues on sync behind the x chunk.
                        m_eng = [nc.scalar, nc.gpsimd, nc.scalar, nc.sync]
                        for m in range(4):
                            eng = m_eng[m]
                            eng.dma_start(out=g_sb[:, m], in_=gu_hb0_m(g_t, m))
                            eng.dma_start(out=u_sb[:, m], in_=gu_hb0_m(u_t, m))
                        d_sb = wp.tile([128, 4, H], BF16, tag="w")
                        nc.scalar.dma_start(out=d_sb, in_=d_hb(d_t, 0))
                    else:
                        g_sb = wp.tile([128, KH, 512], BF16, tag="w")
                        nc.sync.dma_start(out=g_sb, in_=gu_hb(g_t, hb))
                        u_sb = wp.tile([128, KH, 512], BF16, tag="w")
                        nc.sync.dma_start(out=u_sb, in_=gu_hb(u_t, hb))
                        d_sb = wp.tile([128, 4, H], BF16, tag="w")
                        nc.gpsimd.dma_start(out=d_sb, in_=d_hb(d_t, hb))

                    def g_sl(k, m):
                        if g0_mblock:
                            return g_sb[:, m, k, :]
                        return g_sb[:, k, m * 128 : (m + 1) * 128]

                    def u_sl(k, m):
                        if g0_mblock:
                            return u_sb[:, m, k, :]
                        return u_sb[:, k, m * 128 : (m + 1) * 128]

                    if hb == 0 and after_w0 is not None:
                        after_w0()
                    if at_hb is not None and hb in at_hb:
                        at_hb[hb]()
                    for ci, (c_off, cn) in enumerate(chunk_list):
                        x_sb = x_tiles[ci]
                        h_sb = hp.tile([128, 4, cn], BF16, tag="h")
                        if isinstance(x_sb, list):
                            # chunk split over several tiles (k-ranges) so
                            # its x can ride multiple DMA rings at startup
                            x_sl = [t[:, j, :] for t, kn in x_sb
                                    for j in range(kn)]
                        else:
                            x_sl = [x_sb[:, k, :] for k in range(KH)]
                        for m in range(4):
                            pg = ps.tile([128, cn], F32, tag="pg")
                            for k in range(KH):
                                nc.tensor.matmul(
                                    pg, g_sl(k, m), x_sl[k],
                                    start=(k == 0), stop=(k == KH - 1),
                                )
                            nc.scalar.activation(h_sb[:, m, :], pg, SILU)
                            pu = ps.tile([128, cn], F32, tag="pu")
                            for k in range(KH):
                                nc.tensor.matmul(
                                    pu, u_sl(k, m), x_sl[k],
                                    start=(k == 0), stop=(k == KH - 1),
                                )
                            nc.vector.tensor_mul(h_sb[:, m, :], h_sb[:, m, :], pu)
                        for hm in range(KH):
                            pd = ps.tile([128, cn], F32, tag="pd")
                            for k in range(4):
                                nc.tensor.matmul(
                                    pd,
                                    d_sb[:, k, hm * 128 : (hm + 1) * 128],
                                    h_sb[:, k, :],
                                    start=(k == 0), stop=(k == 3),
                                )
                            y_sl = y_sb[:, hm, c_off : c_off + cn]
                            if hb == 0:
                                nc.vector.tensor_copy(y_sl, pd)
                            elif hb < n_hb - 1:
                                nc.vector.tensor_add(y_sl, y_sl, pd)
                            else:
                                yo = op.tile([128, cn], BF16, tag="yo")
                                nc.vector.tensor_add(yo, y_sl, pd)
                                # never the scalar ring: a DMA trigger
                                # waiting for its data blocks the engine
                                # head-of-line, and scalar must keep
                                # running silu
                                eng = nc.sync if hm % 2 == 0 else nc.gpsimd
                                eng.dma_start(
                                    out=y_out_r[:, hm, c_off : c_off + cn],
                                    in_=yo,
                                )

            # ---- x tiles for all three phases, loaded up front.
            # Chunk 0 gates the first matmul, and one ring moves its
            # 1MB only by ~15us (rings measure ~145GB/s effective, not
            # 200): split it k0-5 on sync + k6-7 leading the gpsimd
            # queue so the first matmul fires ~2us earlier.
            K0A = 6
            xa_tiles = [
                [
                    (xp.tile([128, K0A, cn], BF16, tag="xa0a", name="xa0a"),
                     K0A),
                    (xp.tile([128, KH - K0A, cn], BF16, tag="xa0b",
                             name="xa0b"), KH - K0A),
                ]
                if ci == 0
                else xp.tile([128, KH, cn], BF16, tag=f"xa{ci}", name=f"xa{ci}")
                for ci, (_, cn) in enumerate(chs_a)
            ]
            xb_tiles = [
                xp.tile([128, KH, cn], BF16, tag=f"xb{ci}", name=f"xb{ci}")
                for ci, (_, cn) in enumerate(chs_b)
            ]
            xs_tiles = [
                xp.tile([128, KH, cn], BF16, tag=f"xs{ci}", name=f"xs{ci}")
                for ci, (_, cn) in enumerate(chs_s)
            ]
            # first chunk's x: gates the first matmul
            cn0 = chs_a[0][1]
            nc.sync.dma_start(
                out=xa_tiles[0][0][0], in_=x_chunk(xa, 0, cn0, kn=K0A)
            )
            nc.gpsimd.dma_start(
                out=xa_tiles[0][1][0],
                in_=x_chunk(xa, K0A * cn0, cn0, kn=KH - K0A),
            )

            def after_w0():
                # group A's remaining chunks trail on sync behind the odd
                # hb0 m-blocks; with a 512-wide first chunk they land with
                # >15us of margin
                base = KH * chs_a[0][1]
                for ci in range(1, len(chs_a)):
                    cn = chs_a[ci][1]
                    nc.sync.dma_start(out=xa_tiles[ci], in_=x_chunk(xa, base, cn))
                    base += KH * cn

            def load_xs():
                # shared-phase x mid-pass-A on the SYNC ring: sync's queue
                # is busy with per-hb weight loads, so these triggers
                # physically serialize to ~200us.  On an idle ring the
                # scheduler hoists them to the front and the 2.1MB lands
                # in the startup window, oversubscribing the ~358GB/s
                # per-core HBM budget right when the first x chunks and
                # hb0/hb1 weights stream (measured: chunk1 stalls ~4.6us).
                base = 0
                for ci, (_, cn) in enumerate(chs_s):
                    nc.sync.dma_start(
                        out=xs_tiles[ci], in_=x_chunk(xsT, base, cn)
                    )
                    base += KH * cn

            def load_xb():
                base = 0
                for ci, (_, cn) in enumerate(chs_b):
                    nc.sync.dma_start(out=xb_tiles[ci], in_=x_chunk(xb, base, cn))
                    base += KH * cn

            y_a = yp.tile([128, KH, CA], F32, tag="y")

            # next-phase hb0 weights: dedicated tiles on the scalar ring
            # (idle after startup), prefetched with a priority that slots
            # them right after the startup DMAs — the wp pool's rotating
            # loads run just-in-time and the phase transition would stall
            # on them otherwise.  The two transitions share one tag set:
            # the shared-expert generation reuses the pass-B tiles' space
            # once pass B's hb0 has consumed them.
            sw = {}

            def prefetch_w0(gen, g_t, u_t, d_t):
                # dedicated tiles: no pool-rotation WAR wait.  On the SYNC
                # ring (not scalar): an idle engine queue lets the
                # scheduler run these 1MB transfers during the startup HBM
                # crunch (scalar's only other work is activations, and the
                # triggers hoisted to ~14us, delaying d0); sync's steady
                # per-hb weight traffic serializes them to mid-pass.
                g = swp.tile([128, KH, 512], BF16, tag="swg")
                u = swp.tile([128, KH, 512], BF16, tag="swu")
                dd = swp.tile([128, 4, H], BF16, tag="swd")
                nc.sync.dma_start(out=g, in_=gu_hb(g_t, 0))
                nc.sync.dma_start(out=u, in_=gu_hb(u_t, 0))
                nc.sync.dma_start(out=dd, in_=d_hb(d_t, 0))
                sw[gen] = (g, u, dd)

            # ---- expert pass A
            mlp(xa_tiles, chs_a, y_a, wga, wua, wda, I, yTa_r,
                after_w0=after_w0, w0_split=True,
                at_hb={1: load_xb,
                       3: lambda: prefetch_w0("b", wgb, wub, wdb),
                       4: load_xs})

            # ---- expert pass B
            y_b = yp.tile([128, KH, CB], F32, tag="y")
            mlp(xb_tiles, chs_b, y_b, wgb, wub, wdb, I, yTb_r,
                preload0=sw["b"],
                at_hb={2: lambda: prefetch_w0("s", sg, su, sd)})

            # ---- shared-expert phase: this core's 1/8 shard of all tokens
            ys_sb = yp.tile([128, KH, NS], F32, tag="y")
            mlp(xs_tiles, chs_s, ys_sb, sg, su, sd, IS, ysT_r,
                preload0=sw["s"])

    nc.compile()
    return nc


def _pack_gu(w, m_block_hb0=False):
    """[K, N] gate/up weights -> [128, K//128 * N] per-partition-contiguous
    half-block-major blocks (hb0 m-blocked when requested)."""
    K, N = w.shape
    KT = K // 128
    w4 = w.reshape(KT, 128, N // 512, 512).transpose(1, 2, 0, 3)  # p hb k j
    if m_block_hb0:
        hb0 = w4[:, 0].reshape(128, KT, 4, 128).transpose(0, 2, 1, 3)
        return np.ascontiguousarray(
            np.concatenate(
                [hb0.reshape(128, -1), w4[:, 1:].reshape(128, -1)], axis=1
            )
        )
    return np.ascontiguousarray(w4.reshape(128, -1))


def _pack_d(w):
    """[I, H] down weights -> [128, I*H//128] half-block-major blocks."""
    I_, H_ = w.shape
    w4 = w.reshape(I_ // 512, 4, 128, H_).transpose(2, 0, 1, 3)  # p hb t j
    return np.ascontiguousarray(w4.reshape(128, -1))


def _pack_x(xTf, chunks):
    """[H, C] activations -> [128, H//128 * C] chunk-major blocks."""
    H_, C_ = xTf.shape
    xk = xTf.reshape(H_ // 128, 128, C_)
    return np.concatenate(
        [
            xk[:, :, lo : lo + sz].transpose(1, 0, 2).reshape(128, -1)
            for lo, sz in chunks
        ],
        axis=1,
    )


def _install_trace_hook():
    """run_bass_kernel_spmd(trace=True) under axon needs antenv.axon_hooks,
    absent from this image; shim it from trn_agent_boot."""
    import sys
    import types

    if "antenv.axon_hooks" in sys.modules:
        return
    from trn_agent_boot.trn_boot import _ntff_profile_via_ctypes

    hook = _ntff_profile_via_ctypes("/opt/axon/libaxon_pjrt.so")
    mod = types.ModuleType("antenv.axon_hooks")
    mod.get_axon_ntff_profile_hook = lambda: hook
    sys.modules["antenv.axon_hooks"] = mod


def kernel(hidden_states, Wr, Wg, Wu, Wd, Sg, Su, Sd):
    hidden_states = np.asarray(hidden_states, dtype=np.float32)
    Wr = np.asarray(Wr, dtype=np.float32)
    Wg = np.asarray(Wg, dtype=np.float32)
    Wu = np.asarray(Wu, dtype=np.float32)
    Wd = np.asarray(Wd, dtype=np.float32)
    Sg = np.asarray(Sg, dtype=np.float32)
    Su = np.asarray(Su, dtype=np.float32)
    Sd = np.asarray(Sd, dtype=np.float32)

    B, S, H = hidden_states.shape
    E = Wr.shape[1]
    I = Wg.shape[2]
    IS = Sg.shape[1]
    N = B * S
    assert E == N_CORES and N % N_CORES == 0
    NS = N // N_CORES

    flat = hidden_states.reshape(N, H)

    # host router, fp64 (softmax is monotone: top-k by logits == by probs)
    logits = flat.astype(np.float64) @ Wr.astype(np.float64)
    lm = logits.max(axis=1, keepdims=True)
    p = np.exp(logits - lm)
    p /= p.sum(axis=1, keepdims=True)
    order = np.argsort(-logits, axis=1, kind="stable")
    top = order[:, :TOP_K]

    sel = np.zeros((N, E), dtype=bool)
    np.put_along_axis(sel, top, True, axis=1)
    idx_e = [np.flatnonzero(sel[:, e]) for e in range(E)]
    counts = [len(ix) for ix in idx_e]
    CA, CB, a_slots, b_slots = _plan(counts)
    chs_a = _chunks(CA, taper=True)
    chs_b = _chunks(CB)
    chs_s = _chunks(NS)

    flatT = np.ascontiguousarray(flat.T.astype(NP_BF16))  # [H, N] bf16
    Sd_half = (Sd * np.float32(SHARED_SCALE)).astype(NP_BF16)
    sg_p = _pack_gu(Sg.astype(NP_BF16))
    su_p = _pack_gu(Su.astype(NP_BF16))
    sd_p = _pack_d(Sd_half)

    # per-expert weight packs, cached (heavy experts appear in 2 slots)
    gu_cache = {}

    def packed_w(e, m_block):
        key = (e, m_block)
        if key not in gu_cache:
            gu_cache[key] = (
                _pack_gu(Wg[e].astype(NP_BF16), m_block_hb0=m_block),
                _pack_gu(Wu[e].astype(NP_BF16), m_block_hb0=m_block),
            )
        return gu_cache[key]

    d_cache = {}

    def packed_d(e):
        if e not in d_cache:
            d_cache[e] = _pack_d(Wd[e].astype(NP_BF16))
        return d_cache[e]

    def slot_x(slot, cap, chs):
        e, lo, hi = slot
        xT = np.zeros((H, cap), NP_BF16)
        xT[:, : hi - lo] = flatT[:, idx_e[e][lo:hi]]
        return _pack_x(xT, chs)

    in_maps = []
    for c in range(N_CORES):
        ea, eb = a_slots[c][0], b_slots[c][0]
        ga, ua = packed_w(ea, True)
        gb, ub = packed_w(eb, False)
        in_maps.append(
            {
                "xa": slot_x(a_slots[c], CA, chs_a),
                "wga": ga,
                "wua": ua,
                "wda": packed_d(ea),
                "xb": slot_x(b_slots[c], CB, chs_b),
                "wgb": gb,
                "wub": ub,
                "wdb": packed_d(eb),
                "xsT": _pack_x(flatT[:, c * NS : (c + 1) * NS], chs_s),
                "sg": sg_p,
                "su": su_p,
                "sd": sd_p,
            }
        )

    key = (H, I, IS, CA, CB, NS)
    if key not in _NC_CACHE:
        _NC_CACHE[key] = _build(*key)
    nc = _NC_CACHE[key]

    run_kwargs = {}
    if TRACE:
        _install_trace_hook()
        import tempfile

        run_kwargs = {"trace": True, "tmpdir": tempfile.mkdtemp(prefix="moe_trace_")}
    res = run_bass_kernel_spmd(nc, in_maps, core_ids=list(range(N_CORES)), **run_kwargs)
    LAST["exec_time_ns"] = res.exec_time_ns
    LAST["profile_json"] = res.profile_json
    LAST["counts"] = counts
    LAST["C"] = (CA, CB)

    out = np.zeros((N, H), np.float32)
    for c in range(N_CORES):
        for slot, yname in ((a_slots[c], "yTa"), (b_slots[c], "yTb")):
            e, lo, hi = slot
            if hi == lo:
                continue
            ix = idx_e[e][lo:hi]
            w = p[ix, e].astype(np.float32)
            out[ix] += (
                res.results[c][yname][:, : hi - lo].T.astype(np.float32)
                * w[:, None]
            )
        out[c * NS : (c + 1) * NS] += res.results[c]["ysT"].T.astype(np.float32)
    return out.reshape(B, S, H)
